# revision 13
# baseline (speedup 1.0000x reference)
"""Baichuan transformer layer on 8 Trainium2 NeuronCores, tensor-parallel.

Sharding: heads (32 -> 4/core) and MLP intermediate (11008 -> ~1376/core,
padded to 1408) are split across 8 cores. W_pack/gate/up sharded column-wise,
o_proj/down row-wise. ReduceScatter after o_proj (sequence-sharded residual +
RMSNorm), AllGather of the normed bf16 activations, ReduceScatter after
down_proj. Final output is assembled from per-core sequence shards.

Dataflow keeps activations transposed ([hidden, seq]) for all matmuls; the
residual stream stays natural [seq, hidden] in fp32.
"""

import math
import os
import sys

sys.path.insert(0, "/opt/trn_rl_repo")

import numpy as np

import concourse.bass as bass
import concourse.tile as tile
from concourse import bacc, mybir
from concourse.masks import make_identity

P = 128
S = 2048
H = 4096
NKC = H // P            # 32 hidden chunks
NH_LOC = 4              # heads per core
DH = 128
QKV_LOC = NH_LOC * DH   # 512
I_LOC = 1408            # padded local intermediate (11 * 128)
NIT = I_LOC // P        # 11
NST = S // P            # 16 seq tiles
NCH = 4                 # collective chunks
CHS = S // NCH          # 512 tokens per chunk
SHR = CHS // 8          # 64 rows per rank shard per chunk
EPS = 1e-6
SCALE = 1.0 / math.sqrt(DH)
BF = mybir.dt.bfloat16
F32 = mybir.dt.float32

COLL_DT = mybir.dt.float32   # collective dtype

_CACHE = {}


def _build():
    nc = bacc.Bacc("TRN2", target_bir_lowering=False, debug=False, num_devices=8)

    hiddent = nc.dram_tensor("hiddent", [H, S], F32, kind="ExternalInput")
    hidshard = nc.dram_tensor("hidshard", [NCH, SHR, H], F32, kind="ExternalInput")
    maskt = nc.dram_tensor("maskt", [NST, P, P], F32, kind="ExternalInput")
    wpack = nc.dram_tensor("wpack", [H, 3 * QKV_LOC], F32, kind="ExternalInput")
    oproj = nc.dram_tensor("oproj", [QKV_LOC, H], F32, kind="ExternalInput")
    gatew = nc.dram_tensor("gatew", [H, I_LOC], F32, kind="ExternalInput")
    upw = nc.dram_tensor("upw", [H, I_LOC], F32, kind="ExternalInput")
    downw = nc.dram_tensor("downw", [I_LOC, H], F32, kind="ExternalInput")
    ln1 = nc.dram_tensor("ln1", [P, NKC], F32, kind="ExternalInput")
    ln2 = nc.dram_tensor("ln2", [1, H], F32, kind="ExternalInput")
    out = nc.dram_tensor("out", [NCH, SHR, H], F32, kind="ExternalOutput")

    RG = [list(range(8))]
    MUL = mybir.AluOpType.mult
    ADD = mybir.AluOpType.add
    AF = mybir.ActivationFunctionType

    with tile.TileContext(nc) as tc:
      with tc.tile_pool(name="const", bufs=1) as cp, \
           tc.tile_pool(name="dram", bufs=1, space="DRAM") as dp:
        # ---- tiny constants (live whole kernel, ~1KB/partition) ----
        ln1sb = cp.tile([P, NKC], F32)
        nc.sync.dma_start(ln1sb[:], ln1[:])
        ident = cp.tile([P, P], F32)
        make_identity(nc, ident[:])
        ones_bf = cp.tile([P, 1], BF)
        nc.vector.memset(ones_bf[:], 1.0)
        epssb = cp.tile([P, 1], F32)
        nc.vector.memset(epssb[:], EPS)
        rsq_pcol = cp.tile([P, NST], F32)

        # dram scratch
        rsq_d = dp.tile([1, S], F32)
        rs1_in = dp.tile([S, H], COLL_DT)
        rs1_out = dp.tile([NCH, SHR, H], COLL_DT)
        ag_in = dp.tile([NCH, H, SHR], BF)
        ag_out = [dp.tile([8 * H, SHR], BF, addr_space="Shared", name=f"ag_out{c}")
                  for c in range(NCH)]
        rs2_in = dp.tile([S, H], COLL_DT)
        rs2_out = dp.tile([NCH, SHR, H], COLL_DT)
        had_d = dp.tile([NIT, P, S], BF)
        rec_d = dp.tile([NH_LOC, NST, P], F32)
        v_d = dp.tile([NST, P, QKV_LOC], BF)

        # long-lived pools with manual open/close (two-sided allocator)
        atp_cm = tc.tile_pool(name="atp", bufs=1)          # left: attnT p1-p4
        atp = atp_cm.__enter__()
        attnT = atp.tile([P, NH_LOC, S], BF)
        ht_cm = tc.tile_pool(name="ht", bufs=1)            # left: p1-p2
        htp = ht_cm.__enter__()
        ht = htp.tile([P, NKC, S], BF)

        # ==== phase 1: hiddenT load, sumsq, cast*ln1 ====
        with tc.tile_pool(name="hin", bufs=3) as hinp, \
             tc.tile_pool(name="sqp", bufs=2) as sqp, \
             tc.tile_pool(name="ssps", bufs=1, space="PSUM") as ssp, \
             tc.tile_pool(name="smal", bufs=1) as smp:
            ss = ssp.tile([1, S], F32)
            for k in range(NKC):
                hf = hinp.tile([P, S], F32, tag="hf")
                nc.sync.dma_start(hf[:], hiddent[k * P:(k + 1) * P, :])
                sq = sqp.tile([P, S], BF, tag="sq")
                nc.scalar.activation(sq[:], hf[:], AF.Square)
                for j in range(4):
                    nc.tensor.matmul(
                        ss[:, j * 512:(j + 1) * 512], ones_bf[:],
                        sq[:, j * 512:(j + 1) * 512],
                        start=(k == 0), stop=(k == NKC - 1))
                nc.vector.tensor_tensor(
                    ht[:, k, :], hf[:],
                    ln1sb[:, k:k + 1].to_broadcast((P, S)), MUL)
            # rsq = 1/sqrt(mean + eps)
            std = smp.tile([1, S], F32, tag="std")
            nc.scalar.activation(std[:], ss[:], AF.Sqrt,
                                 bias=epssb[:1, :], scale=1.0 / H)
            rsq = smp.tile([1, S], F32, tag="rsq")
            nc.vector.reciprocal(rsq[:], std[:])
            nc.sync.dma_start(rsq_d[:], rsq[:])
            nc.sync.dma_start(
                rsq_pcol[:], rsq_d.rearrange("o (n p) -> p (o n)", p=P))

        # right side: qkv outputs, live p2-p3
        qkv_cm = tc.tile_pool(name="qkv", bufs=1, side="right")
        qkvp = qkv_cm.__enter__()
        qT = qkvp.tile([P, NH_LOC, S], BF)
        kT = qkvp.tile([P, NH_LOC, S], BF)
        rsq_bc = qkvp.tile([P, S], BF)
        nc.gpsimd.dma_start(rsq_bc[:], rsq_d[:].to_broadcast((P, S)))

        # ==== phase 2: QKV projections ====
        with tc.tile_pool(name="wst", bufs=2) as wsp, \
             tc.tile_pool(name="qps", bufs=1, space="PSUM") as qpsp:
            for part in range(2):       # 0 = q, 1 = k
                dst = qT if part == 0 else kT
                for h in range(NH_LOC):
                    wcol = wsp.tile([P, NKC, P], BF, tag="wcol")
                    col0 = part * QKV_LOC + h * DH
                    nc.gpsimd.dma_start(
                        wcol[:],
                        wpack.rearrange("(k p) c -> p k c", p=P)
                        [:, :, col0:col0 + DH])
                    ps = [qpsp.tile([P, 512], F32, tag=f"qk{j}", name=f"qk{j}")
                          for j in range(4)]
                    for k in range(NKC):
                        for j in range(4):
                            nc.tensor.matmul(
                                ps[j][:], wcol[:, k, :],
                                ht[:, k, j * 512:(j + 1) * 512],
                                start=(k == 0), stop=(k == NKC - 1))
                    for j in range(4):
                        nc.vector.tensor_tensor(
                            dst[:, h, j * 512:(j + 1) * 512], ps[j][:],
                            rsq_bc[:, j * 512:(j + 1) * 512], MUL)
            # v in natural [s, d] layout (lhsT = hT chunk), staged to DRAM
            for vg in range(2):
                ps = [qpsp.tile([P, 512], F32, tag=f"qk{j}", name=f"vq{j}")
                      for j in range(4)] + \
                     [qpsp.tile([P, 512], F32, tag=f"v{j}", name=f"v{j}")
                      for j in range(4)]
                for k in range(NKC):
                    wv = wsp.tile([P, QKV_LOC], BF, tag="wv")
                    nc.gpsimd.dma_start(
                        wv[:], wpack[k * P:(k + 1) * P,
                                     2 * QKV_LOC:3 * QKV_LOC])
                    for sti in range(8):
                        st = vg * 8 + sti
                        nc.tensor.matmul(
                            ps[sti][:], ht[:, k, st * P:(st + 1) * P],
                            wv[:], start=(k == 0), stop=(k == NKC - 1))
                for sti in range(8):
                    st = vg * 8 + sti
                    vstg = wsp.tile([P, QKV_LOC], BF, tag="vstg")
                    nc.scalar.activation(
                        vstg[:], ps[sti][:], AF.Copy,
                        scale=rsq_pcol[:, st:st + 1])
                    nc.sync.dma_start(v_d[st], vstg[:])

        ht_cm.__exit__(None, None, None)   # free 128KB/part

        # ==== phase 3: attention ====
        with tc.tile_pool(name="msk", bufs=1) as mkp, \
             tc.tile_pool(name="probs", bufs=4) as prp, \
             tc.tile_pool(name="vh", bufs=2) as vhp, \
             tc.tile_pool(name="scps", bufs=2, space="PSUM") as scp, \
             tc.tile_pool(name="atps", bufs=1, space="PSUM") as apsp, \
             tc.tile_pool(name="attmisc", bufs=2) as amp:
            masksb = mkp.tile([P, NST, P], F32)
            nc.sync.dma_start(masksb[:], maskt.rearrange("n k q -> k n q"))
            v_r = v_d.rearrange("st p c -> p st c")
            for h in range(NH_LOC):
                vh = vhp.tile([P, NST, DH], BF, tag="vh")
                nc.sync.dma_start(vh[:], v_r[:, :, h * DH:(h + 1) * DH])
                aps = apsp.tile([P, S], F32, tag="aps", name="aps")
                sps = apsp.tile([P, NST], F32, tag="sps", name="sps")
                for kb in range(NST):
                    q0 = kb * P
                    pt = prp.tile([P, S], BF, tag="probs", name="pt")
                    bnds = []
                    a = q0
                    while a < S:
                        b = min((a // 512 + 1) * 512, S)
                        bnds.append((a, b))
                        a = b
                    for (a, b) in bnds:
                        sc = scp.tile([P, 512], F32, tag="sc", name="sc")
                        nc.tensor.matmul(
                            sc[:, :b - a], kT[:, h, q0:q0 + P],
                            qT[:, h, a:b], start=True, stop=True)
                        if a == q0:
                            nc.vector.tensor_tensor(
                                sc[:, :P], sc[:, :P], masksb[:, kb, :], ADD)
                        nc.scalar.activation(
                            pt[:, a:b], sc[:, :b - a], AF.Exp, scale=SCALE)
                    for (a, b) in bnds:
                        nc.tensor.matmul(
                            aps[:, a:b], vh[:, kb, :], pt[:, a:b],
                            start=(kb == 0), stop=(kb == (b - 1) // P))
                    for qb in range(kb, NST):
                        # single bank shared by 16 accumulation chains:
                        # only the very first matmul may clear the bank
                        nc.tensor.matmul(
                            sps[:, qb:qb + 1], pt[:, qb * P:(qb + 1) * P],
                            ones_bf[:], start=(kb == 0 and qb == 0),
                            stop=(kb == qb), skip_group_check=True)
                rec = amp.tile([P, NST], F32, tag="rec")
                nc.vector.reciprocal(rec[:], sps[:])
                rtp = apsp.tile([NST, P], F32, tag="rtp", name="rtp")
                nc.tensor.transpose(rtp[:], rec[:], ident[:])
                rts = amp.tile([NST, P], F32, tag="rts")
                nc.scalar.copy(rts[:], rtp[:])
                nc.sync.dma_start(rec_d[h], rts[:])
                rbc = amp.tile([P, S], F32, tag="rbc")
                nc.gpsimd.dma_start(
                    rbc[:],
                    rec_d[h].rearrange("a b -> (a b)")[None, :]
                    .to_broadcast((P, S)))
                nc.vector.tensor_tensor(attnT[:, h, :], aps[:], rbc[:], MUL)

        qkv_cm.__exit__(None, None, None)

        # ==== phase 4: o_proj + RS1 ====
        with tc.tile_pool(name="opj", bufs=1) as opp, \
             tc.tile_pool(name="ops", bufs=1, space="PSUM") as opsp, \
             tc.tile_pool(name="ost", bufs=2) as ostp:
            ow = opp.tile([P, NH_LOC, H], BF)
            for h in range(NH_LOC):
                nc.gpsimd.dma_start(ow[:, h, :], oproj[h * P:(h + 1) * P, :])
            for st in range(NST):
                ps8 = [opsp.tile([P, 512], F32, tag=f"o{j}", name=f"o{j}")
                       for j in range(8)]
                for h in range(NH_LOC):
                    for j in range(8):
                        nc.tensor.matmul(
                            ps8[j][:], attnT[:, h, st * P:(st + 1) * P],
                            ow[:, h, j * 512:(j + 1) * 512],
                            start=(h == 0), stop=(h == NH_LOC - 1))
                osb = ostp.tile([P, H], COLL_DT, tag="osb")
                for j in range(8):
                    if j % 2 == 0:
                        nc.vector.tensor_copy(
                            osb[:, j * 512:(j + 1) * 512], ps8[j][:])
                    else:
                        nc.scalar.copy(
                            osb[:, j * 512:(j + 1) * 512], ps8[j][:])
                nc.sync.dma_start(rs1_in[st * P:(st + 1) * P, :], osb[:])
                if st % 4 == 3:
                    c = st // 4
                    nc.gpsimd.collective_compute(
                        "ReduceScatter", ADD, replica_groups=RG,
                        ins=[rs1_in[c * CHS:(c + 1) * CHS, :].opt()],
                        outs=[rs1_out[c].opt()])

        atp_cm.__exit__(None, None, None)

        # residual stream shards, live to the end (right side)
        h2_cm = tc.tile_pool(name="h2", bufs=1, side="right")
        h2p = h2_cm.__enter__()
        h2pk = [h2p.tile([P, H], F32, tag=f"h2_{j}", name=f"h2_{j}")
                for j in range(NCH // 2)]

        def h2sl(c):
            return h2pk[c // 2][(c % 2) * SHR:(c % 2) * SHR + SHR, :]

        mt_cm = tc.tile_pool(name="mt", bufs=1)
        mtp = mt_cm.__enter__()
        mT = mtp.tile([P, NKC, S], BF)

        # ==== phase 5: residual + rmsnorm + AG, per chunk ====
        with tc.tile_pool(name="chk", bufs=1) as chp:
            ln2bc = chp.tile([P, H], BF, tag="ln2bc")
            nc.gpsimd.dma_start(ln2bc[:], ln2[:].to_broadcast((P, H)))
            for c in range(NCH):
                b = (c % 2) * SHR
                h2c = h2sl(c)
                nc.sync.dma_start(h2c, hidshard[c])
                tmp = chp.tile([P, H], F32, tag="tmp")
                nc.sync.dma_start(tmp[b:b + SHR, :], rs1_out[c])
                nc.vector.tensor_tensor(h2c, h2c, tmp[b:b + SHR, :], ADD)
                sq2 = chp.tile([P, H], BF, tag="msh", name="sq2")
                nc.scalar.activation(sq2[b:b + SHR, :], h2c, AF.Square)
                var = chp.tile([P, 1], F32, tag="var")
                nc.vector.reduce_sum(var[b:b + SHR, :], sq2[b:b + SHR, :],
                                     axis=mybir.AxisListType.X)
                std2 = chp.tile([P, 1], F32, tag="std2")
                nc.scalar.activation(std2[b:b + SHR, :], var[b:b + SHR, :],
                                     AF.Sqrt, bias=epssb[b:b + SHR, :],
                                     scale=1.0 / H)
                rst = chp.tile([P, 1], F32, tag="rst")
                nc.vector.reciprocal(rst[b:b + SHR, :], std2[b:b + SHR, :])
                mtm = chp.tile([P, H], BF, tag="mtm")
                nc.scalar.activation(mtm[b:b + SHR, :], h2c, AF.Copy,
                                     scale=rst[b:b + SHR, :])
                msh = chp.tile([P, H], BF, tag="msh")
                nc.vector.tensor_tensor(msh[b:b + SHR, :], mtm[b:b + SHR, :],
                                        ln2bc[b:b + SHR, :], MUL)
                mts = chp.tile([P, NKC, SHR], BF, tag="mts")
                nc.sync.dma_start_transpose(mts[:], msh[b:b + SHR, :])
                nc.sync.dma_start(
                    ag_in[c].rearrange("(ks p) n -> p ks n", p=P), mts[:])
                nc.gpsimd.collective_compute(
                    "AllGather", mybir.AluOpType.bypass, replica_groups=RG,
                    ins=[ag_in[c].opt()], outs=[ag_out[c].opt()])
                gsrc = ag_out[c].rearrange("(r ks p) n -> ks p r n", r=8, p=P)
                for k in range(NKC):
                    nc.sync.dma_start(
                        mT[:, k, c * CHS:(c + 1) * CHS]
                        .rearrange("p (r n) -> p r n", r=8), gsrc[k])

        # ==== phase 6: gate/up + silu ====
        with tc.tile_pool(name="gst", bufs=1) as gsp, \
             tc.tile_pool(name="gwa", bufs=2) as gwap, \
             tc.tile_pool(name="gwb", bufs=1) as gwbp, \
             tc.tile_pool(name="gps", bufs=1, space="PSUM") as gpsp:
            gw_r = gatew.rearrange("(k p) c -> p k c", p=P)
            uw_r = upw.rearrange("(k p) c -> p k c", p=P)
            for i in range(NIT):
                gcol = gwap.tile([P, NKC, P], BF, tag="gcol")
                nc.gpsimd.dma_start(gcol[:], gw_r[:, :, i * P:(i + 1) * P])
                ucol = gwbp.tile([P, NKC, P], BF, tag="ucol")
                nc.gpsimd.dma_start(ucol[:], uw_r[:, :, i * P:(i + 1) * P])
                gps = [gpsp.tile([P, 512], F32, tag=f"g{j}", name=f"g{j}")
                       for j in range(4)]
                ups = [gpsp.tile([P, 512], F32, tag=f"u{j}", name=f"u{j}")
                       for j in range(4)]
                for k in range(NKC):
                    for j in range(4):
                        nc.tensor.matmul(
                            gps[j][:], gcol[:, k, :],
                            mT[:, k, j * 512:(j + 1) * 512],
                            start=(k == 0), stop=(k == NKC - 1))
                    for j in range(4):
                        nc.tensor.matmul(
                            ups[j][:], ucol[:, k, :],
                            mT[:, k, j * 512:(j + 1) * 512],
                            start=(k == 0), stop=(k == NKC - 1))
                gs = gsp.tile([P, S], BF, tag="gs")
                us = gsp.tile([P, S], BF, tag="us")
                for j in range(4):
                    nc.scalar.activation(
                        gs[:, j * 512:(j + 1) * 512], gps[j][:], AF.Silu)
                    nc.vector.tensor_copy(
                        us[:, j * 512:(j + 1) * 512], ups[j][:])
                hadt = gsp.tile([P, S], BF, tag="hadt")
                nc.vector.tensor_tensor(hadt[:], gs[:], us[:], MUL)
                nc.sync.dma_start(had_d[i], hadt[:])

        mt_cm.__exit__(None, None, None)

        # ==== phase 7: down proj + RS2 ====
        with tc.tile_pool(name="dw", bufs=1) as dwp, \
             tc.tile_pool(name="dst", bufs=2) as dsp, \
             tc.tile_pool(name="hst", bufs=3) as hsp, \
             tc.tile_pool(name="dps", bufs=1, space="PSUM") as dpsp:
            dw = dwp.tile([P, NIT, H], BF)
            for i in range(NIT):
                nc.gpsimd.dma_start(dw[:, i, :], downw[i * P:(i + 1) * P, :])
            had_r = had_d.rearrange("i p s -> p i s")
            for st in range(NST):
                hads = hsp.tile([P, NIT, P], BF, tag="hads")
                nc.sync.dma_start(hads[:], had_r[:, :, st * P:(st + 1) * P])
                ps8 = [dpsp.tile([P, 512], F32, tag=f"d{j}", name=f"d{j}")
                       for j in range(8)]
                for i in range(NIT):
                    for j in range(8):
                        nc.tensor.matmul(
                            ps8[j][:], hads[:, i, :],
                            dw[:, i, j * 512:(j + 1) * 512],
                            start=(i == 0), stop=(i == NIT - 1))
                dsb = dsp.tile([P, H], COLL_DT, tag="dsb")
                for j in range(8):
                    if j % 2 == 0:
                        nc.vector.tensor_copy(
                            dsb[:, j * 512:(j + 1) * 512], ps8[j][:])
                    else:
                        nc.scalar.copy(
                            dsb[:, j * 512:(j + 1) * 512], ps8[j][:])
                nc.sync.dma_start(rs2_in[st * P:(st + 1) * P, :], dsb[:])
                if st % 4 == 3:
                    c = st // 4
                    nc.gpsimd.collective_compute(
                        "ReduceScatter", ADD, replica_groups=RG,
                        ins=[rs2_in[c * CHS:(c + 1) * CHS, :].opt()],
                        outs=[rs2_out[c].opt()])
            # ==== phase 8: final residual ====
            with tc.tile_pool(name="fin", bufs=1) as fpp:
                for c in range(NCH):
                    b = (c % 2) * SHR
                    f1 = fpp.tile([P, H], F32, tag="f1")
                    nc.sync.dma_start(f1[b:b + SHR, :], rs2_out[c])
                    fo = fpp.tile([P, H], F32, tag="fo")
                    nc.vector.tensor_tensor(fo[b:b + SHR, :], f1[b:b + SHR, :],
                                            h2sl(c), ADD)
                    nc.sync.dma_start(out[c], fo[b:b + SHR, :])

        h2_cm.__exit__(None, None, None)

    nc.finalize()
    return nc


def _prep_inputs(hidden_states, attention_mask, W_pack, o_proj, gate_w, up_w,
                 down_w, ln1_w, ln2_w):
    """Slice/layout full inputs into 8 per-core input dicts."""
    hs = np.ascontiguousarray(np.asarray(hidden_states, dtype=np.float32)[0])
    hiddent = np.ascontiguousarray(hs.T)                      # [H, S]
    mask = np.asarray(attention_mask, dtype=np.float32)[0, 0]  # [S, S]
    masktd = np.stack([
        np.ascontiguousarray(mask[b * P:(b + 1) * P, b * P:(b + 1) * P].T)
        for b in range(NST)])                                  # [NST, P, P]
    W_pack = np.asarray(W_pack, dtype=np.float32)
    o_proj = np.asarray(o_proj, dtype=np.float32)
    gate_w = np.asarray(gate_w, dtype=np.float32)
    up_w = np.asarray(up_w, dtype=np.float32)
    down_w = np.asarray(down_w, dtype=np.float32)
    ln1 = np.ascontiguousarray(
        np.asarray(ln1_w, dtype=np.float32).reshape(NKC, P).T)  # [P, NKC]
    ln2 = np.asarray(ln2_w, dtype=np.float32).reshape(1, H)

    # intermediate split: 6 cores x 1408 + 2 cores x 1280 (padded to 1408)
    i_sizes = [1408] * 6 + [1280] * 2
    i_offs = np.cumsum([0] + i_sizes)

    in_maps = []
    for r in range(8):
        q0 = r * QKV_LOC
        wp = np.concatenate([
            W_pack[:, q0:q0 + QKV_LOC],
            W_pack[:, H + q0:H + q0 + QKV_LOC],
            W_pack[:, 2 * H + q0:2 * H + q0 + QKV_LOC]], axis=1)
        opl = o_proj[q0:q0 + QKV_LOC, :]
        io0, io1 = i_offs[r], i_offs[r + 1]
        isz = io1 - io0
        gl = np.zeros((H, I_LOC), np.float32)
        gl[:, :isz] = gate_w[:, io0:io1]
        ul = np.zeros((H, I_LOC), np.float32)
        ul[:, :isz] = up_w[:, io0:io1]
        dl = np.zeros((I_LOC, H), np.float32)
        dl[:isz, :] = down_w[io0:io1, :]
        hsh = np.stack([
            hs[c * CHS + r * SHR: c * CHS + (r + 1) * SHR, :]
            for c in range(NCH)])                              # [NCH, SHR, H]
        in_maps.append({
            "hiddent": hiddent,
            "hidshard": np.ascontiguousarray(hsh),
            "maskt": masktd,
            "wpack": np.ascontiguousarray(wp),
            "oproj": np.ascontiguousarray(opl),
            "gatew": gl,
            "upw": ul,
            "downw": dl,
            "ln1": ln1,
            "ln2": ln2,
        })
    return in_maps


def _assemble(results):
    """results[r]['out'] is [NCH, SHR, H]; reassemble [1, S, H]."""
    full = np.empty((S, H), np.float32)
    for r in range(8):
        o = results[r]["out"]
        for c in range(NCH):
            full[c * CHS + r * SHR: c * CHS + (r + 1) * SHR, :] = o[c]
    return full[None]


def _get_nc():
    if "nc" not in _CACHE:
        _CACHE["nc"] = _build()
    return _CACHE["nc"]


def kernel(**inputs):
    from concourse.bass_utils import run_bass_kernel_spmd
    nc = _get_nc()
    in_maps = _prep_inputs(**inputs)
    res = run_bass_kernel_spmd(nc, in_maps, core_ids=list(range(8)))
    return _assemble(res.results)


if __name__ == "__main__":
    rng = np.random.RandomState(0)
    ins = {
        "hidden_states": rng.randn(1, S, H).astype(np.float32),
        "attention_mask": np.where(
            np.tril(np.ones((S, S), bool)), 0.0,
            np.finfo(np.float32).min)[None, None].astype(np.float32),
        "W_pack": rng.randn(H, 3 * H).astype(np.float32) * 0.02,
        "o_proj": rng.randn(H, H).astype(np.float32) * 0.02,
        "gate_w": rng.randn(H, 11008).astype(np.float32) * 0.02,
        "up_w": rng.randn(H, 11008).astype(np.float32) * 0.02,
        "down_w": rng.randn(11008, H).astype(np.float32) * 0.02,
        "ln1_w": np.ones(H, np.float32),
        "ln2_w": np.ones(H, np.float32),
    }
    out = kernel(**ins)
    print("kernel output", out.shape, out.dtype, float(np.abs(out).mean()))


# revision 21
# speedup vs baseline: 1.6899x; 1.6899x over previous
"""Baichuan transformer layer on 8 Trainium2 NeuronCores, tensor-parallel.

Sharding: heads (32 -> 4/core) and MLP intermediate (11008 -> ~1376/core,
padded to 1408) are split across 8 cores. W_pack/gate/up sharded column-wise,
o_proj/down row-wise. ReduceScatter after o_proj (sequence-sharded residual +
RMSNorm), AllGather of the normed bf16 activations, ReduceScatter after
down_proj. Final output is assembled from per-core sequence shards.

Dataflow keeps activations transposed ([hidden, seq]) for all matmuls; the
residual stream stays natural [seq, hidden] in fp32.
"""

import math
import os
import sys

sys.path.insert(0, "/opt/trn_rl_repo")

import ml_dtypes
import numpy as np

import concourse.bass as bass
import concourse.tile as tile
from concourse import bacc, mybir
from concourse.masks import make_identity

P = 128
S = 2048
H = 4096
NKC = H // P            # 32 hidden chunks
NH_LOC = 4              # heads per core
DH = 128
QKV_LOC = NH_LOC * DH   # 512
I_LOC = 1408            # padded local intermediate (11 * 128)
NIT = I_LOC // P        # 11
NST = S // P            # 16 seq tiles
NCH = 4                 # collective chunks
CHS = S // NCH          # 512 tokens per chunk
SHR = CHS // 8          # 64 rows per rank shard per chunk
EPS = 1e-6
SCALE = 1.0 / math.sqrt(DH)
BF = mybir.dt.bfloat16
F32 = mybir.dt.float32

COLL_DT = mybir.dt.float32   # collective dtype

_CACHE = {}


def _build():
    nc = bacc.Bacc("TRN2", target_bir_lowering=False, debug=False, num_devices=8)

    hiddent = nc.dram_tensor("hiddent", [H, S], F32, kind="ExternalInput")
    hidshard = nc.dram_tensor("hidshard", [NCH, SHR, H], F32, kind="ExternalInput")
    maskt = nc.dram_tensor("maskt", [NST, P, P], F32, kind="ExternalInput")
    wpack = nc.dram_tensor("wpack", [H, 3 * QKV_LOC], BF, kind="ExternalInput")
    oproj = nc.dram_tensor("oproj", [QKV_LOC, H], BF, kind="ExternalInput")
    gatew = nc.dram_tensor("gatew", [H, I_LOC], BF, kind="ExternalInput")
    upw = nc.dram_tensor("upw", [H, I_LOC], BF, kind="ExternalInput")
    downw = nc.dram_tensor("downw", [I_LOC, H], BF, kind="ExternalInput")
    ln1 = nc.dram_tensor("ln1", [P, NKC], F32, kind="ExternalInput")
    ln2 = nc.dram_tensor("ln2", [1, H], F32, kind="ExternalInput")
    out = nc.dram_tensor("out", [NCH, SHR, H], F32, kind="ExternalOutput")

    RG = [list(range(8))]
    MUL = mybir.AluOpType.mult
    ADD = mybir.AluOpType.add
    AF = mybir.ActivationFunctionType

    with tile.TileContext(nc) as tc:
      with tc.tile_pool(name="const", bufs=1) as cp, \
           tc.tile_pool(name="dram", bufs=1, space="DRAM") as dp:
        # ---- tiny constants (live whole kernel, ~1KB/partition) ----
        ln1sb = cp.tile([P, NKC], F32)
        nc.sync.dma_start(ln1sb[:], ln1[:])
        ident = cp.tile([P, P], F32)
        make_identity(nc, ident[:])
        ones_bf = cp.tile([P, 1], BF)
        nc.vector.memset(ones_bf[:], 1.0)
        epssb = cp.tile([P, 1], F32)
        nc.vector.memset(epssb[:], EPS)
        rsq_pcol = cp.tile([P, NST], F32)

        # dram scratch
        rsq_d = dp.tile([1, S], F32)
        rs1_in = [dp.tile([CHS, H], COLL_DT, name=f"rs1_in{c}")
                  for c in range(NCH)]
        rs1_out = dp.tile([NCH, SHR, H], COLL_DT)
        ag_in = dp.tile([NCH, H, SHR], BF)
        ag_out = [dp.tile([8 * H, SHR], BF, addr_space="Shared", name=f"ag_out{c}")
                  for c in range(NCH)]
        rs2_in = [dp.tile([CHS, H], COLL_DT, name=f"rs2_in{c}")
                  for c in range(NCH)]
        rs2_out = dp.tile([NCH, SHR, H], COLL_DT)
        had_d = dp.tile([NIT, P, S], BF)
        rec_d = dp.tile([NH_LOC, NST, P], F32)
        v_d = dp.tile([NST, P, QKV_LOC], BF)

        # long-lived pools with manual open/close (two-sided allocator)
        atp_cm = tc.tile_pool(name="atp", bufs=1)          # left: attnT p1-p4
        atp = atp_cm.__enter__()
        attnT = atp.tile([P, NH_LOC, S], BF)
        ht_cm = tc.tile_pool(name="ht", bufs=1)            # left: p1-p2
        htp = ht_cm.__enter__()
        ht = htp.tile([P, NKC, S], BF)

        # ==== phase 1: hiddenT load, sumsq, cast*ln1 ====
        with tc.tile_pool(name="hin", bufs=3) as hinp, \
             tc.tile_pool(name="sqp", bufs=2) as sqp, \
             tc.tile_pool(name="ssps", bufs=1, space="PSUM") as ssp, \
             tc.tile_pool(name="smal", bufs=1) as smp:
            ss = ssp.tile([1, S], F32)
            for k in range(NKC):
                hf = hinp.tile([P, S], F32, tag="hf")
                nc.sync.dma_start(hf[:], hiddent[k * P:(k + 1) * P, :])
                sq = sqp.tile([P, S], BF, tag="sq")
                nc.scalar.activation(sq[:], hf[:], AF.Square)
                for j in range(4):
                    nc.tensor.matmul(
                        ss[:, j * 512:(j + 1) * 512], ones_bf[:],
                        sq[:, j * 512:(j + 1) * 512],
                        start=(k == 0), stop=(k == NKC - 1))
                nc.vector.tensor_tensor(
                    ht[:, k, :], hf[:],
                    ln1sb[:, k:k + 1].to_broadcast((P, S)), MUL)
            # rsq = 1/sqrt(mean + eps)
            std = smp.tile([1, S], F32, tag="std")
            nc.scalar.activation(std[:], ss[:], AF.Sqrt,
                                 bias=epssb[:1, :], scale=1.0 / H)
            rsq = smp.tile([1, S], F32, tag="rsq")
            nc.vector.reciprocal(rsq[:], std[:])
            nc.sync.dma_start(rsq_d[:], rsq[:])
            nc.sync.dma_start(
                rsq_pcol[:], rsq_d.rearrange("o (n p) -> p (o n)", p=P))

        # right side: qkv outputs, live p2-p3
        qkv_cm = tc.tile_pool(name="qkv", bufs=1, side="right")
        qkvp = qkv_cm.__enter__()
        qT = qkvp.tile([P, NH_LOC, S], BF)
        kT = qkvp.tile([P, NH_LOC, S], BF)
        rsq_bc = qkvp.tile([P, S], BF)
        nc.gpsimd.dma_start(rsq_bc[:], rsq_d[:].to_broadcast((P, S)))

        # ==== phase 2: QKV projections ====
        with tc.tile_pool(name="wst", bufs=2) as wsp, \
             tc.tile_pool(name="qps", bufs=1, space="PSUM") as qpsp:
            for part in range(2):       # 0 = q, 1 = k
                dst = qT if part == 0 else kT
                for h in range(NH_LOC):
                    wcol = wsp.tile([P, NKC, P], BF, tag="wcol")
                    col0 = part * QKV_LOC + h * DH
                    nc.scalar.dma_start(
                        wcol[:],
                        wpack.rearrange("(k p) c -> p k c", p=P)
                        [:, :, col0:col0 + DH])
                    ps = [qpsp.tile([P, 512], F32, tag=f"qk{j}", name=f"qk{j}")
                          for j in range(4)]
                    for k in range(NKC):
                        for j in range(4):
                            nc.tensor.matmul(
                                ps[j][:], wcol[:, k, :],
                                ht[:, k, j * 512:(j + 1) * 512],
                                start=(k == 0), stop=(k == NKC - 1))
                    for j in range(4):
                        nc.vector.tensor_tensor(
                            dst[:, h, j * 512:(j + 1) * 512], ps[j][:],
                            rsq_bc[:, j * 512:(j + 1) * 512], MUL)
            # v in natural [s, d] layout (lhsT = hT chunk), staged to DRAM
            for vg in range(2):
                ps = [qpsp.tile([P, 512], F32, tag=f"qk{j}", name=f"vq{j}")
                      for j in range(4)] + \
                     [qpsp.tile([P, 512], F32, tag=f"v{j}", name=f"v{j}")
                      for j in range(4)]
                for k in range(NKC):
                    wv = wsp.tile([P, QKV_LOC], BF, tag="wv")
                    nc.scalar.dma_start(
                        wv[:], wpack[k * P:(k + 1) * P,
                                     2 * QKV_LOC:3 * QKV_LOC])
                    for sti in range(8):
                        st = vg * 8 + sti
                        nc.tensor.matmul(
                            ps[sti][:], ht[:, k, st * P:(st + 1) * P],
                            wv[:], start=(k == 0), stop=(k == NKC - 1))
                for sti in range(8):
                    st = vg * 8 + sti
                    vstg = wsp.tile([P, QKV_LOC], BF, tag="vstg")
                    nc.scalar.activation(
                        vstg[:], ps[sti][:], AF.Copy,
                        scale=rsq_pcol[:, st:st + 1])
                    nc.sync.dma_start(v_d[st], vstg[:])

        ht_cm.__exit__(None, None, None)   # free 128KB/part

        # prefetch o_proj weights during attention
        opj_cm = tc.tile_pool(name="opj", bufs=1)
        opp = opj_cm.__enter__()
        ow = opp.tile([P, NH_LOC, H], BF)
        for h in range(NH_LOC):
            nc.scalar.dma_start(ow[:, h, :], oproj[h * P:(h + 1) * P, :])

        # ==== phase 3: attention ====
        with tc.tile_pool(name="msk", bufs=1) as mkp, \
             tc.tile_pool(name="probs", bufs=4) as prp, \
             tc.tile_pool(name="vh", bufs=2) as vhp, \
             tc.tile_pool(name="scps", bufs=2, space="PSUM") as scp, \
             tc.tile_pool(name="atps", bufs=1, space="PSUM") as apsp, \
             tc.tile_pool(name="attmisc", bufs=2) as amp:
            masksb = mkp.tile([P, NST, P], F32)
            nc.sync.dma_start(masksb[:], maskt.rearrange("n k q -> k n q"))
            v_r = v_d.rearrange("st p c -> p st c")
            for h in range(NH_LOC):
                vh = vhp.tile([P, NST, DH], BF, tag="vh")
                nc.sync.dma_start(vh[:], v_r[:, :, h * DH:(h + 1) * DH])
                aps = apsp.tile([P, S], F32, tag="aps", name="aps")
                sps = apsp.tile([P, NST], F32, tag="sps", name="sps")
                for kb in range(NST):
                    q0 = kb * P
                    pt = prp.tile([P, S], BF, tag="probs", name="pt")
                    bnds = []
                    a = q0
                    while a < S:
                        b = min((a // 512 + 1) * 512, S)
                        bnds.append((a, b))
                        a = b
                    for (a, b) in bnds:
                        sc = scp.tile([P, 512], F32, tag="sc", name="sc")
                        nc.tensor.matmul(
                            sc[:, :b - a], kT[:, h, q0:q0 + P],
                            qT[:, h, a:b], start=True, stop=True)
                        if a == q0:
                            nc.vector.tensor_tensor(
                                sc[:, :P], sc[:, :P], masksb[:, kb, :], ADD)
                        nc.scalar.activation(
                            pt[:, a:b], sc[:, :b - a], AF.Exp, scale=SCALE)
                    for (a, b) in bnds:
                        nc.tensor.matmul(
                            aps[:, a:b], vh[:, kb, :], pt[:, a:b],
                            start=(kb == 0), stop=(kb == (b - 1) // P))
                    for qb in range(kb, NST):
                        # single bank shared by 16 accumulation chains:
                        # only the very first matmul may clear the bank
                        nc.tensor.matmul(
                            sps[:, qb:qb + 1], pt[:, qb * P:(qb + 1) * P],
                            ones_bf[:], start=(kb == 0 and qb == 0),
                            stop=(kb == qb), skip_group_check=True)
                rec = amp.tile([P, NST], F32, tag="rec")
                nc.vector.reciprocal(rec[:], sps[:])
                rtp = apsp.tile([NST, P], F32, tag="rtp", name="rtp")
                nc.tensor.transpose(rtp[:], rec[:], ident[:])
                rts = amp.tile([NST, P], F32, tag="rts")
                nc.scalar.copy(rts[:], rtp[:])
                nc.sync.dma_start(rec_d[h], rts[:])
                rbc = amp.tile([P, S], F32, tag="rbc")
                nc.gpsimd.dma_start(
                    rbc[:],
                    rec_d[h].rearrange("a b -> (a b)")[None, :]
                    .to_broadcast((P, S)))
                nc.vector.tensor_tensor(attnT[:, h, :], aps[:], rbc[:], MUL)

        qkv_cm.__exit__(None, None, None)

        # residual stream shards, live to the end (right side)
        h2_cm = tc.tile_pool(name="h2", bufs=1, side="right")
        h2p = h2_cm.__enter__()
        h2pk = [h2p.tile([P, H], F32, tag=f"h2_{j}", name=f"h2_{j}")
                for j in range(NCH // 2)]

        def h2sl(c):
            return h2pk[c // 2][(c % 2) * SHR:(c % 2) * SHR + SHR, :]

        # ==== phase 4: o_proj + per-chunk [RS1 -> norm -> AG] ====
        with tc.tile_pool(name="ops", bufs=1, space="PSUM") as opsp, \
             tc.tile_pool(name="ost", bufs=2) as ostp, \
             tc.tile_pool(name="chk", bufs=1) as chp:
            ln2bc = chp.tile([P, H], BF, tag="ln2bc")
            nc.gpsimd.dma_start(ln2bc[:], ln2[:].to_broadcast((P, H)))
            for st in range(NST):
                ps8 = [opsp.tile([P, 512], F32, tag=f"o{j}", name=f"o{j}")
                       for j in range(8)]
                for h in range(NH_LOC):
                    for j in range(8):
                        nc.tensor.matmul(
                            ps8[j][:], attnT[:, h, st * P:(st + 1) * P],
                            ow[:, h, j * 512:(j + 1) * 512],
                            start=(h == 0), stop=(h == NH_LOC - 1))
                osb = ostp.tile([P, H], COLL_DT, tag="osb")
                for j in range(8):
                    if j % 2 == 0:
                        nc.vector.tensor_copy(
                            osb[:, j * 512:(j + 1) * 512], ps8[j][:])
                    else:
                        nc.scalar.copy(
                            osb[:, j * 512:(j + 1) * 512], ps8[j][:])
                nc.sync.dma_start(
                    rs1_in[st // 4][(st % 4) * P:(st % 4 + 1) * P, :], osb[:])
                if st % 4 == 3:
                    c = st // 4
                    nc.gpsimd.collective_compute(
                        "ReduceScatter", ADD, replica_groups=RG,
                        ins=[rs1_in[c][:].opt()],
                        outs=[rs1_out[c].opt()])
            # per-chunk residual + rmsnorm + AllGather, emitted after the
            # o_proj loop so their RS1-waits don't block engine queues
            for c in range(NCH):
                b = (c % 2) * SHR
                h2c = h2sl(c)
                nc.sync.dma_start(h2c, hidshard[c])
                tmp = chp.tile([P, H], F32, tag="tmp")
                nc.sync.dma_start(tmp[b:b + SHR, :], rs1_out[c])
                nc.vector.tensor_tensor(h2c, h2c, tmp[b:b + SHR, :], ADD)
                sq2 = chp.tile([P, H], BF, tag="msh", name="sq2")
                nc.scalar.activation(sq2[b:b + SHR, :], h2c, AF.Square)
                var = chp.tile([P, 1], F32, tag="var")
                nc.vector.reduce_sum(var[b:b + SHR, :], sq2[b:b + SHR, :],
                                     axis=mybir.AxisListType.X)
                std2 = chp.tile([P, 1], F32, tag="std2")
                nc.scalar.activation(std2[b:b + SHR, :], var[b:b + SHR, :],
                                     AF.Sqrt, bias=epssb[b:b + SHR, :],
                                     scale=1.0 / H)
                rst = chp.tile([P, 1], F32, tag="rst")
                nc.vector.reciprocal(rst[b:b + SHR, :], std2[b:b + SHR, :])
                mtm = chp.tile([P, H], BF, tag="mtm")
                nc.scalar.activation(mtm[b:b + SHR, :], h2c, AF.Copy,
                                     scale=rst[b:b + SHR, :])
                msh = chp.tile([P, H], BF, tag="msh")
                nc.vector.tensor_tensor(msh[b:b + SHR, :], mtm[b:b + SHR, :],
                                        ln2bc[b:b + SHR, :], MUL)
                mts = chp.tile([P, NKC, SHR], BF, tag="mts")
                nc.sync.dma_start_transpose(mts[:], msh[b:b + SHR, :])
                nc.sync.dma_start(
                    ag_in[c].rearrange("(ks p) n -> p ks n", p=P), mts[:])
                nc.gpsimd.collective_compute(
                    "AllGather", mybir.AluOpType.bypass, replica_groups=RG,
                    ins=[ag_in[c].opt()], outs=[ag_out[c].opt()])

        opj_cm.__exit__(None, None, None)
        atp_cm.__exit__(None, None, None)

        mt_cm = tc.tile_pool(name="mt", bufs=1)
        mtp = mt_cm.__enter__()
        mT = mtp.tile([P, NKC, S], BF)

        # ==== phase 6: gate/up + silu (chunk-outer: overlap with AG pipeline) ====
        with tc.tile_pool(name="gst", bufs=2) as gsp, \
             tc.tile_pool(name="gwa", bufs=2) as gwap, \
             tc.tile_pool(name="gwb", bufs=2) as gwbp, \
             tc.tile_pool(name="gps", bufs=1, space="PSUM") as gpsp:
            gw_r = gatew.rearrange("(k p) c -> p k c", p=P)
            uw_r = upw.rearrange("(k p) c -> p k c", p=P)
            for c in range(NCH):
                c0 = c * CHS
                for r in range(8):
                    nc.sync.dma_start(
                        mT[:, :, c0 + r * SHR:c0 + (r + 1) * SHR],
                        ag_out[c][r * H:(r + 1) * H, :]
                        .rearrange("(ks p) n -> p ks n", p=P))
                for i in range(NIT):
                    gcol = gwap.tile([P, NKC, P], BF, tag="gcol")
                    nc.scalar.dma_start(gcol[:], gw_r[:, :, i * P:(i + 1) * P])
                    ucol = gwbp.tile([P, NKC, P], BF, tag="ucol")
                    nc.scalar.dma_start(ucol[:], uw_r[:, :, i * P:(i + 1) * P])
                    gp = gpsp.tile([P, 512], F32, tag=f"g{i % 4}", name="gp")
                    up = gpsp.tile([P, 512], F32, tag=f"u{i % 4}", name="up")
                    for k in range(NKC):
                        nc.tensor.matmul(
                            gp[:], gcol[:, k, :], mT[:, k, c0:c0 + CHS],
                            start=(k == 0), stop=(k == NKC - 1))
                        nc.tensor.matmul(
                            up[:], ucol[:, k, :], mT[:, k, c0:c0 + CHS],
                            start=(k == 0), stop=(k == NKC - 1))
                    gs = gsp.tile([P, CHS], BF, tag="gs")
                    us = gsp.tile([P, CHS], BF, tag="us")
                    nc.scalar.activation(gs[:], gp[:], AF.Silu)
                    nc.vector.tensor_copy(us[:], up[:])
                    hadt = gsp.tile([P, CHS], BF, tag="hadt")
                    nc.vector.tensor_tensor(hadt[:], gs[:], us[:], MUL)
                    nc.sync.dma_start(had_d[i][:, c0:c0 + CHS], hadt[:])

        mt_cm.__exit__(None, None, None)

        # ==== phase 7: down proj + RS2 ====
        with tc.tile_pool(name="dw", bufs=1) as dwp, \
             tc.tile_pool(name="dst", bufs=2) as dsp, \
             tc.tile_pool(name="hst", bufs=3) as hsp, \
             tc.tile_pool(name="dps", bufs=1, space="PSUM") as dpsp:
            dw = dwp.tile([P, NIT, H], BF)
            for i in range(NIT):
                nc.scalar.dma_start(dw[:, i, :], downw[i * P:(i + 1) * P, :])
            had_r = had_d.rearrange("i p s -> p i s")
            for st in range(NST):
                hads = hsp.tile([P, NIT, P], BF, tag="hads")
                nc.sync.dma_start(hads[:], had_r[:, :, st * P:(st + 1) * P])
                ps8 = [dpsp.tile([P, 512], F32, tag=f"d{j}", name=f"d{j}")
                       for j in range(8)]
                for i in range(NIT):
                    for j in range(8):
                        nc.tensor.matmul(
                            ps8[j][:], hads[:, i, :],
                            dw[:, i, j * 512:(j + 1) * 512],
                            start=(i == 0), stop=(i == NIT - 1))
                dsb = dsp.tile([P, H], COLL_DT, tag="dsb")
                for j in range(8):
                    if j % 2 == 0:
                        nc.vector.tensor_copy(
                            dsb[:, j * 512:(j + 1) * 512], ps8[j][:])
                    else:
                        nc.scalar.copy(
                            dsb[:, j * 512:(j + 1) * 512], ps8[j][:])
                nc.sync.dma_start(
                    rs2_in[st // 4][(st % 4) * P:(st % 4 + 1) * P, :], dsb[:])
                if st % 4 == 3:
                    c = st // 4
                    nc.gpsimd.collective_compute(
                        "ReduceScatter", ADD, replica_groups=RG,
                        ins=[rs2_in[c][:].opt()],
                        outs=[rs2_out[c].opt()])
            # ==== phase 8: final residual ====
            with tc.tile_pool(name="fin", bufs=1) as fpp:
                for c in range(NCH):
                    b = (c % 2) * SHR
                    f1 = fpp.tile([P, H], F32, tag="f1")
                    nc.sync.dma_start(f1[b:b + SHR, :], rs2_out[c])
                    fo = fpp.tile([P, H], F32, tag="fo")
                    nc.vector.tensor_tensor(fo[b:b + SHR, :], f1[b:b + SHR, :],
                                            h2sl(c), ADD)
                    nc.sync.dma_start(out[c], fo[b:b + SHR, :])

        h2_cm.__exit__(None, None, None)

    nc.finalize()
    return nc


def _prep_inputs(hidden_states, attention_mask, W_pack, o_proj, gate_w, up_w,
                 down_w, ln1_w, ln2_w):
    """Slice/layout full inputs into 8 per-core input dicts."""
    hs = np.ascontiguousarray(np.asarray(hidden_states, dtype=np.float32)[0])
    hiddent = np.ascontiguousarray(hs.T)                      # [H, S]
    mask = np.asarray(attention_mask, dtype=np.float32)[0, 0]  # [S, S]
    masktd = np.stack([
        np.ascontiguousarray(mask[b * P:(b + 1) * P, b * P:(b + 1) * P].T)
        for b in range(NST)])                                  # [NST, P, P]
    W_pack = np.asarray(W_pack, dtype=np.float32)
    o_proj = np.asarray(o_proj, dtype=np.float32)
    gate_w = np.asarray(gate_w, dtype=np.float32)
    up_w = np.asarray(up_w, dtype=np.float32)
    down_w = np.asarray(down_w, dtype=np.float32)
    ln1 = np.ascontiguousarray(
        np.asarray(ln1_w, dtype=np.float32).reshape(NKC, P).T)  # [P, NKC]
    ln2 = np.asarray(ln2_w, dtype=np.float32).reshape(1, H)

    # intermediate split: 6 cores x 1408 + 2 cores x 1280 (padded to 1408)
    i_sizes = [1408] * 6 + [1280] * 2
    i_offs = np.cumsum([0] + i_sizes)

    in_maps = []
    for r in range(8):
        q0 = r * QKV_LOC
        wp = np.concatenate([
            W_pack[:, q0:q0 + QKV_LOC],
            W_pack[:, H + q0:H + q0 + QKV_LOC],
            W_pack[:, 2 * H + q0:2 * H + q0 + QKV_LOC]], axis=1)
        opl = o_proj[q0:q0 + QKV_LOC, :]
        io0, io1 = i_offs[r], i_offs[r + 1]
        isz = io1 - io0
        gl = np.zeros((H, I_LOC), np.float32)
        gl[:, :isz] = gate_w[:, io0:io1]
        ul = np.zeros((H, I_LOC), np.float32)
        ul[:, :isz] = up_w[:, io0:io1]
        dl = np.zeros((I_LOC, H), np.float32)
        dl[:isz, :] = down_w[io0:io1, :]
        hsh = np.stack([
            hs[c * CHS + r * SHR: c * CHS + (r + 1) * SHR, :]
            for c in range(NCH)])                              # [NCH, SHR, H]
        bf = ml_dtypes.bfloat16
        in_maps.append({
            "hiddent": hiddent,
            "hidshard": np.ascontiguousarray(hsh),
            "maskt": masktd,
            "wpack": np.ascontiguousarray(wp).astype(bf),
            "oproj": np.ascontiguousarray(opl).astype(bf),
            "gatew": gl.astype(bf),
            "upw": ul.astype(bf),
            "downw": dl.astype(bf),
            "ln1": ln1,
            "ln2": ln2,
        })
    return in_maps


def _assemble(results):
    """results[r]['out'] is [NCH, SHR, H]; reassemble [1, S, H]."""
    full = np.empty((S, H), np.float32)
    for r in range(8):
        o = results[r]["out"]
        for c in range(NCH):
            full[c * CHS + r * SHR: c * CHS + (r + 1) * SHR, :] = o[c]
    return full[None]


def _get_nc():
    if "nc" not in _CACHE:
        _CACHE["nc"] = _build()
    return _CACHE["nc"]


def kernel(**inputs):
    from concourse.bass_utils import run_bass_kernel_spmd
    nc = _get_nc()
    in_maps = _prep_inputs(**inputs)
    res = run_bass_kernel_spmd(nc, in_maps, core_ids=list(range(8)))
    return _assemble(res.results)


if __name__ == "__main__":
    rng = np.random.RandomState(0)
    ins = {
        "hidden_states": rng.randn(1, S, H).astype(np.float32),
        "attention_mask": np.where(
            np.tril(np.ones((S, S), bool)), 0.0,
            np.finfo(np.float32).min)[None, None].astype(np.float32),
        "W_pack": rng.randn(H, 3 * H).astype(np.float32) * 0.02,
        "o_proj": rng.randn(H, H).astype(np.float32) * 0.02,
        "gate_w": rng.randn(H, 11008).astype(np.float32) * 0.02,
        "up_w": rng.randn(H, 11008).astype(np.float32) * 0.02,
        "down_w": rng.randn(11008, H).astype(np.float32) * 0.02,
        "ln1_w": np.ones(H, np.float32),
        "ln2_w": np.ones(H, np.float32),
    }
    out = kernel(**ins)
    print("kernel output", out.shape, out.dtype, float(np.abs(out).mean()))


# revision 23
# speedup vs baseline: 12063.2224x; 7138.4733x over previous
"""Baichuan transformer layer on 8 Trainium2 NeuronCores, tensor-parallel.

Sharding: heads (32 -> 4/core) and MLP intermediate (11008 -> ~1376/core,
padded to 1408) are split across 8 cores. W_pack/gate/up sharded column-wise,
o_proj/down row-wise. ReduceScatter after o_proj (sequence-sharded residual +
RMSNorm), AllGather of the normed bf16 activations, ReduceScatter after
down_proj. Final output is assembled from per-core sequence shards.

Dataflow keeps activations transposed ([hidden, seq]) for all matmuls; the
residual stream stays natural [seq, hidden] in fp32.
"""

import math
import os
import sys

sys.path.insert(0, "/opt/trn_rl_repo")

import ml_dtypes
import numpy as np

import concourse.bass as bass
import concourse.tile as tile
from concourse import bacc, mybir
from concourse.masks import make_identity

P = 128
S = 2048
H = 4096
NKC = H // P            # 32 hidden chunks
NH_LOC = 4              # heads per core
DH = 128
QKV_LOC = NH_LOC * DH   # 512
I_LOC = 1408            # padded local intermediate (11 * 128)
NIT = I_LOC // P        # 11
NST = S // P            # 16 seq tiles
NCH = 4                 # collective chunks
CHS = S // NCH          # 512 tokens per chunk
SHR = CHS // 8          # 64 rows per rank shard per chunk
EPS = 1e-6
SCALE = 1.0 / math.sqrt(DH)
BF = mybir.dt.bfloat16
F32 = mybir.dt.float32

COLL_DT = mybir.dt.float32   # collective dtype

_CACHE = {}


def _build():
    nc = bacc.Bacc("TRN2", target_bir_lowering=False, debug=False, num_devices=8)

    hiddent = nc.dram_tensor("hiddent", [H, S], F32, kind="ExternalInput")
    hidshard = nc.dram_tensor("hidshard", [NCH, SHR, H], F32, kind="ExternalInput")
    maskt = nc.dram_tensor("maskt", [NST, P, P], F32, kind="ExternalInput")
    wpack = nc.dram_tensor("wpack", [H, 3 * QKV_LOC], BF, kind="ExternalInput")
    oproj = nc.dram_tensor("oproj", [QKV_LOC, H], BF, kind="ExternalInput")
    gatew = nc.dram_tensor("gatew", [H, I_LOC], BF, kind="ExternalInput")
    upw = nc.dram_tensor("upw", [H, I_LOC], BF, kind="ExternalInput")
    downw = nc.dram_tensor("downw", [I_LOC, H], BF, kind="ExternalInput")
    ln1 = nc.dram_tensor("ln1", [P, NKC], F32, kind="ExternalInput")
    ln2 = nc.dram_tensor("ln2", [1, H], F32, kind="ExternalInput")
    out = nc.dram_tensor("out", [NCH, SHR, H], F32, kind="ExternalOutput")

    RG = [list(range(8))]
    MUL = mybir.AluOpType.mult
    ADD = mybir.AluOpType.add
    AF = mybir.ActivationFunctionType

    with tile.TileContext(nc) as tc:
      with tc.tile_pool(name="const", bufs=1) as cp, \
           tc.tile_pool(name="dram", bufs=1, space="DRAM") as dp:
        # ---- tiny constants (live whole kernel, ~1KB/partition) ----
        ln1sb = cp.tile([P, NKC], F32)
        nc.sync.dma_start(ln1sb[:], ln1[:])
        ident = cp.tile([P, P], F32)
        make_identity(nc, ident[:])
        ones_bf = cp.tile([P, 1], BF)
        nc.vector.memset(ones_bf[:], 1.0)
        epssb = cp.tile([P, 1], F32)
        nc.vector.memset(epssb[:], EPS)
        rsq_pcol = cp.tile([P, NST], F32)

        # dram scratch
        rsq_d = dp.tile([1, S], F32)
        rs1_in = [dp.tile([CHS, H], COLL_DT, name=f"rs1_in{c}")
                  for c in range(NCH)]
        rs1_out = dp.tile([NCH, SHR, H], COLL_DT)
        ag_in = dp.tile([NCH, H, SHR], BF)
        ag_out = [dp.tile([8 * H, SHR], BF, addr_space="Shared", name=f"ag_out{c}")
                  for c in range(NCH)]
        rs2_in = [dp.tile([CHS, H], COLL_DT, name=f"rs2_in{c}")
                  for c in range(NCH)]
        rs2_out = dp.tile([NCH, SHR, H], COLL_DT)
        had_d = dp.tile([NIT, P, S], BF)
        rec_d = dp.tile([NH_LOC, NST, P], F32)
        v_d = dp.tile([NST, P, QKV_LOC], BF)

        # long-lived pools with manual open/close (two-sided allocator)
        atp_cm = tc.tile_pool(name="atp", bufs=1)          # left: attnT p1-p4
        atp = atp_cm.__enter__()
        attnT = atp.tile([P, NH_LOC, S], BF)
        ht_cm = tc.tile_pool(name="ht", bufs=1)            # left: p1-p2
        htp = ht_cm.__enter__()
        ht = htp.tile([P, NKC, S], BF)

        # ==== phase 1: hiddenT load, sumsq, cast*ln1 ====
        with tc.tile_pool(name="hin", bufs=3) as hinp, \
             tc.tile_pool(name="sqp", bufs=2) as sqp, \
             tc.tile_pool(name="ssps", bufs=1, space="PSUM") as ssp, \
             tc.tile_pool(name="smal", bufs=1) as smp:
            ss = ssp.tile([1, S], F32)
            for k in range(NKC):
                hf = hinp.tile([P, S], F32, tag="hf")
                nc.sync.dma_start(hf[:], hiddent[k * P:(k + 1) * P, :])
                sq = sqp.tile([P, S], BF, tag="sq")
                nc.scalar.activation(sq[:], hf[:], AF.Square)
                for j in range(4):
                    nc.tensor.matmul(
                        ss[:, j * 512:(j + 1) * 512], ones_bf[:],
                        sq[:, j * 512:(j + 1) * 512],
                        start=(k == 0), stop=(k == NKC - 1))
                nc.vector.tensor_tensor(
                    ht[:, k, :], hf[:],
                    ln1sb[:, k:k + 1].to_broadcast((P, S)), MUL)
            # rsq = 1/sqrt(mean + eps)
            std = smp.tile([1, S], F32, tag="std")
            nc.scalar.activation(std[:], ss[:], AF.Sqrt,
                                 bias=epssb[:1, :], scale=1.0 / H)
            rsq = smp.tile([1, S], F32, tag="rsq")
            nc.vector.reciprocal(rsq[:], std[:])
            nc.sync.dma_start(rsq_d[:], rsq[:])
            nc.sync.dma_start(
                rsq_pcol[:], rsq_d.rearrange("o (n p) -> p (o n)", p=P))

        # right side: qkv outputs, live p2-p3
        qkv_cm = tc.tile_pool(name="qkv", bufs=1, side="right")
        qkvp = qkv_cm.__enter__()
        qT = qkvp.tile([P, NH_LOC, S], BF)
        kT = qkvp.tile([P, NH_LOC, S], BF)
        rsq_bc = qkvp.tile([P, S], BF)
        nc.gpsimd.dma_start(rsq_bc[:], rsq_d[:].to_broadcast((P, S)))

        # ==== phase 2: QKV projections ====
        with tc.tile_pool(name="wst", bufs=2) as wsp, \
             tc.tile_pool(name="qps", bufs=1, space="PSUM") as qpsp:
            for part in range(2):       # 0 = q, 1 = k
                dst = qT if part == 0 else kT
                for h in range(NH_LOC):
                    wcol = wsp.tile([P, NKC, P], BF, tag="wcol")
                    col0 = part * QKV_LOC + h * DH
                    nc.scalar.dma_start(
                        wcol[:],
                        wpack.rearrange("(k p) c -> p k c", p=P)
                        [:, :, col0:col0 + DH])
                    ps = [qpsp.tile([P, 512], F32, tag=f"qk{j}", name=f"qk{j}")
                          for j in range(4)]
                    for k in range(NKC):
                        for j in range(4):
                            nc.tensor.matmul(
                                ps[j][:], wcol[:, k, :],
                                ht[:, k, j * 512:(j + 1) * 512],
                                start=(k == 0), stop=(k == NKC - 1))
                    for j in range(4):
                        nc.vector.tensor_tensor(
                            dst[:, h, j * 512:(j + 1) * 512], ps[j][:],
                            rsq_bc[:, j * 512:(j + 1) * 512], MUL)
            # v in natural [s, d] layout (lhsT = hT chunk), staged to DRAM
            for vg in range(2):
                ps = [qpsp.tile([P, 512], F32, tag=f"qk{j}", name=f"vq{j}")
                      for j in range(4)] + \
                     [qpsp.tile([P, 512], F32, tag=f"v{j}", name=f"v{j}")
                      for j in range(4)]
                for k in range(NKC):
                    wv = wsp.tile([P, QKV_LOC], BF, tag="wv")
                    nc.scalar.dma_start(
                        wv[:], wpack[k * P:(k + 1) * P,
                                     2 * QKV_LOC:3 * QKV_LOC])
                    for sti in range(8):
                        st = vg * 8 + sti
                        nc.tensor.matmul(
                            ps[sti][:], ht[:, k, st * P:(st + 1) * P],
                            wv[:], start=(k == 0), stop=(k == NKC - 1))
                for sti in range(8):
                    st = vg * 8 + sti
                    vstg = wsp.tile([P, QKV_LOC], BF, tag="vstg")
                    nc.scalar.activation(
                        vstg[:], ps[sti][:], AF.Copy,
                        scale=rsq_pcol[:, st:st + 1])
                    nc.sync.dma_start(v_d[st], vstg[:])

        ht_cm.__exit__(None, None, None)   # free 128KB/part

        # prefetch o_proj weights during attention
        opj_cm = tc.tile_pool(name="opj", bufs=1)
        opp = opj_cm.__enter__()
        ow = opp.tile([P, NH_LOC, H], BF)
        for h in range(NH_LOC):
            nc.scalar.dma_start(ow[:, h, :], oproj[h * P:(h + 1) * P, :])

        # ==== phase 3: attention ====
        with tc.tile_pool(name="msk", bufs=1) as mkp, \
             tc.tile_pool(name="probs", bufs=4) as prp, \
             tc.tile_pool(name="vh", bufs=2) as vhp, \
             tc.tile_pool(name="scps", bufs=2, space="PSUM") as scp, \
             tc.tile_pool(name="atps", bufs=1, space="PSUM") as apsp, \
             tc.tile_pool(name="attmisc", bufs=2) as amp:
            masksb = mkp.tile([P, NST, P], F32)
            nc.sync.dma_start(masksb[:], maskt.rearrange("n k q -> k n q"))
            v_r = v_d.rearrange("st p c -> p st c")
            for h in range(NH_LOC):
                vh = vhp.tile([P, NST, DH], BF, tag="vh")
                nc.sync.dma_start(vh[:], v_r[:, :, h * DH:(h + 1) * DH])
                aps = apsp.tile([P, S], F32, tag="aps", name="aps")
                sps = apsp.tile([P, NST], F32, tag="sps", name="sps")
                for kb in range(NST):
                    q0 = kb * P
                    pt = prp.tile([P, S], BF, tag="probs", name="pt")
                    bnds = []
                    a = q0
                    while a < S:
                        b = min((a // 512 + 1) * 512, S)
                        bnds.append((a, b))
                        a = b
                    for (a, b) in bnds:
                        sc = scp.tile([P, 512], F32, tag="sc", name="sc")
                        nc.tensor.matmul(
                            sc[:, :b - a], kT[:, h, q0:q0 + P],
                            qT[:, h, a:b], start=True, stop=True)
                        if a == q0:
                            nc.vector.tensor_tensor(
                                sc[:, :P], sc[:, :P], masksb[:, kb, :], ADD)
                        nc.scalar.activation(
                            pt[:, a:b], sc[:, :b - a], AF.Exp, scale=SCALE)
                    for (a, b) in bnds:
                        nc.tensor.matmul(
                            aps[:, a:b], vh[:, kb, :], pt[:, a:b],
                            start=(kb == 0), stop=(kb == (b - 1) // P))
                    for qb in range(kb, NST):
                        # single bank shared by 16 accumulation chains:
                        # only the very first matmul may clear the bank
                        nc.tensor.matmul(
                            sps[:, qb:qb + 1], pt[:, qb * P:(qb + 1) * P],
                            ones_bf[:], start=(kb == 0 and qb == 0),
                            stop=(kb == qb), skip_group_check=True)
                rec = amp.tile([P, NST], F32, tag="rec")
                nc.vector.reciprocal(rec[:], sps[:])
                rtp = apsp.tile([NST, P], F32, tag="rtp", name="rtp")
                nc.tensor.transpose(rtp[:], rec[:], ident[:])
                rts = amp.tile([NST, P], F32, tag="rts")
                nc.scalar.copy(rts[:], rtp[:])
                nc.sync.dma_start(rec_d[h], rts[:])
                rbc = amp.tile([P, S], F32, tag="rbc")
                nc.gpsimd.dma_start(
                    rbc[:],
                    rec_d[h].rearrange("a b -> (a b)")[None, :]
                    .to_broadcast((P, S)))
                nc.vector.tensor_tensor(attnT[:, h, :], aps[:], rbc[:], MUL)

        qkv_cm.__exit__(None, None, None)

        # residual stream shards, live to the end (right side)
        h2_cm = tc.tile_pool(name="h2", bufs=1, side="right")
        h2p = h2_cm.__enter__()
        h2pk = [h2p.tile([P, H], F32, tag=f"h2_{j}", name=f"h2_{j}")
                for j in range(NCH // 2)]

        def h2sl(c):
            return h2pk[c // 2][(c % 2) * SHR:(c % 2) * SHR + SHR, :]

        # ==== phase 4: o_proj + per-chunk [RS1 -> norm -> AG] ====
        with tc.tile_pool(name="ops", bufs=1, space="PSUM") as opsp, \
             tc.tile_pool(name="ost", bufs=2) as ostp, \
             tc.tile_pool(name="chk", bufs=1) as chp:
            ln2bc = chp.tile([P, H], BF, tag="ln2bc")
            nc.gpsimd.dma_start(ln2bc[:], ln2[:].to_broadcast((P, H)))
            for st in range(NST):
                ps8 = [opsp.tile([P, 512], F32, tag=f"o{j}", name=f"o{j}")
                       for j in range(8)]
                for h in range(NH_LOC):
                    for j in range(8):
                        nc.tensor.matmul(
                            ps8[j][:], attnT[:, h, st * P:(st + 1) * P],
                            ow[:, h, j * 512:(j + 1) * 512],
                            start=(h == 0), stop=(h == NH_LOC - 1))
                osb = ostp.tile([P, H], COLL_DT, tag="osb")
                for j in range(8):
                    if j % 2 == 0:
                        nc.vector.tensor_copy(
                            osb[:, j * 512:(j + 1) * 512], ps8[j][:])
                    else:
                        nc.scalar.copy(
                            osb[:, j * 512:(j + 1) * 512], ps8[j][:])
                nc.sync.dma_start(
                    rs1_in[st // 4][(st % 4) * P:(st % 4 + 1) * P, :], osb[:])
                if st % 4 == 3:
                    c = st // 4
                    nc.gpsimd.collective_compute(
                        "ReduceScatter", ADD, replica_groups=RG,
                        ins=[rs1_in[c][:].opt()],
                        outs=[rs1_out[c].opt()])
            # per-chunk residual + rmsnorm + AllGather, emitted after the
            # o_proj loop so their RS1-waits don't block engine queues
            for c in range(NCH):
                b = (c % 2) * SHR
                h2c = h2sl(c)
                nc.sync.dma_start(h2c, hidshard[c])
                tmp = chp.tile([P, H], F32, tag="tmp")
                nc.sync.dma_start(tmp[b:b + SHR, :], rs1_out[c])
                nc.vector.tensor_tensor(h2c, h2c, tmp[b:b + SHR, :], ADD)
                sq2 = chp.tile([P, H], BF, tag="msh", name="sq2")
                nc.scalar.activation(sq2[b:b + SHR, :], h2c, AF.Square)
                var = chp.tile([P, 1], F32, tag="var")
                nc.vector.reduce_sum(var[b:b + SHR, :], sq2[b:b + SHR, :],
                                     axis=mybir.AxisListType.X)
                std2 = chp.tile([P, 1], F32, tag="std2")
                nc.scalar.activation(std2[b:b + SHR, :], var[b:b + SHR, :],
                                     AF.Sqrt, bias=epssb[b:b + SHR, :],
                                     scale=1.0 / H)
                rst = chp.tile([P, 1], F32, tag="rst")
                nc.vector.reciprocal(rst[b:b + SHR, :], std2[b:b + SHR, :])
                mtm = chp.tile([P, H], BF, tag="mtm")
                nc.scalar.activation(mtm[b:b + SHR, :], h2c, AF.Copy,
                                     scale=rst[b:b + SHR, :])
                msh = chp.tile([P, H], BF, tag="msh")
                nc.vector.tensor_tensor(msh[b:b + SHR, :], mtm[b:b + SHR, :],
                                        ln2bc[b:b + SHR, :], MUL)
                mts = chp.tile([P, NKC, SHR], BF, tag="mts")
                nc.sync.dma_start_transpose(mts[:], msh[b:b + SHR, :])
                nc.sync.dma_start(
                    ag_in[c].rearrange("(ks p) n -> p ks n", p=P), mts[:])
                nc.gpsimd.collective_compute(
                    "AllGather", mybir.AluOpType.bypass, replica_groups=RG,
                    ins=[ag_in[c].opt()], outs=[ag_out[c].opt()])

        opj_cm.__exit__(None, None, None)
        atp_cm.__exit__(None, None, None)

        mt_cm = tc.tile_pool(name="mt", bufs=1)
        mtp = mt_cm.__enter__()
        mT = mtp.tile([P, NKC, S], BF)

        # ==== phase 6: gate/up + silu (chunk-outer: overlap with AG pipeline) ====
        with tc.tile_pool(name="gst", bufs=2) as gsp, \
             tc.tile_pool(name="gwa", bufs=2) as gwap, \
             tc.tile_pool(name="gwb", bufs=2) as gwbp, \
             tc.tile_pool(name="gps", bufs=1, space="PSUM") as gpsp:
            gw_r = gatew.rearrange("(k p) c -> p k c", p=P)
            uw_r = upw.rearrange("(k p) c -> p k c", p=P)
            for c in range(NCH):
                c0 = c * CHS
                for r in range(8):
                    nc.sync.dma_start(
                        mT[:, :, c0 + r * SHR:c0 + (r + 1) * SHR],
                        ag_out[c][r * H:(r + 1) * H, :]
                        .rearrange("(ks p) n -> p ks n", p=P))
                for i in range(NIT):
                    gcol = gwap.tile([P, NKC, P], BF, tag="gcol")
                    nc.scalar.dma_start(gcol[:], gw_r[:, :, i * P:(i + 1) * P])
                    ucol = gwbp.tile([P, NKC, P], BF, tag="ucol")
                    nc.scalar.dma_start(ucol[:], uw_r[:, :, i * P:(i + 1) * P])
                    gp = gpsp.tile([P, 512], F32, tag=f"g{i % 4}", name="gp")
                    up = gpsp.tile([P, 512], F32, tag=f"u{i % 4}", name="up")
                    for k in range(NKC):
                        nc.tensor.matmul(
                            gp[:], gcol[:, k, :], mT[:, k, c0:c0 + CHS],
                            start=(k == 0), stop=(k == NKC - 1))
                        nc.tensor.matmul(
                            up[:], ucol[:, k, :], mT[:, k, c0:c0 + CHS],
                            start=(k == 0), stop=(k == NKC - 1))
                    gs = gsp.tile([P, CHS], BF, tag="gs")
                    us = gsp.tile([P, CHS], BF, tag="us")
                    nc.scalar.activation(gs[:], gp[:], AF.Silu)
                    nc.vector.tensor_copy(us[:], up[:])
                    hadt = gsp.tile([P, CHS], BF, tag="hadt")
                    nc.vector.tensor_tensor(hadt[:], gs[:], us[:], MUL)
                    nc.sync.dma_start(had_d[i][:, c0:c0 + CHS], hadt[:])

        mt_cm.__exit__(None, None, None)

        # ==== phase 7: down proj + RS2 ====
        with tc.tile_pool(name="dw", bufs=1) as dwp, \
             tc.tile_pool(name="dst", bufs=2) as dsp, \
             tc.tile_pool(name="hst", bufs=3) as hsp, \
             tc.tile_pool(name="dps", bufs=1, space="PSUM") as dpsp:
            dw = dwp.tile([P, NIT, H], BF)
            for i in range(NIT):
                nc.scalar.dma_start(dw[:, i, :], downw[i * P:(i + 1) * P, :])
            had_r = had_d.rearrange("i p s -> p i s")
            for st in range(NST):
                hads = hsp.tile([P, NIT, P], BF, tag="hads")
                nc.sync.dma_start(hads[:], had_r[:, :, st * P:(st + 1) * P])
                ps8 = [dpsp.tile([P, 512], F32, tag=f"d{j}", name=f"d{j}")
                       for j in range(8)]
                for i in range(NIT):
                    for j in range(8):
                        nc.tensor.matmul(
                            ps8[j][:], hads[:, i, :],
                            dw[:, i, j * 512:(j + 1) * 512],
                            start=(i == 0), stop=(i == NIT - 1))
                dsb = dsp.tile([P, H], COLL_DT, tag="dsb")
                for j in range(8):
                    if j % 2 == 0:
                        nc.vector.tensor_copy(
                            dsb[:, j * 512:(j + 1) * 512], ps8[j][:])
                    else:
                        nc.scalar.copy(
                            dsb[:, j * 512:(j + 1) * 512], ps8[j][:])
                nc.sync.dma_start(
                    rs2_in[st // 4][(st % 4) * P:(st % 4 + 1) * P, :], dsb[:])
                if st % 4 == 3:
                    c = st // 4
                    nc.gpsimd.collective_compute(
                        "ReduceScatter", ADD, replica_groups=RG,
                        ins=[rs2_in[c][:].opt()],
                        outs=[rs2_out[c].opt()])
            # ==== phase 8: final residual ====
            with tc.tile_pool(name="fin", bufs=1) as fpp:
                for c in range(NCH):
                    b = (c % 2) * SHR
                    f1 = fpp.tile([P, H], F32, tag="f1")
                    nc.sync.dma_start(f1[b:b + SHR, :], rs2_out[c])
                    fo = fpp.tile([P, H], F32, tag="fo")
                    nc.vector.tensor_tensor(fo[b:b + SHR, :], f1[b:b + SHR, :],
                                            h2sl(c), ADD)
                    nc.sync.dma_start(out[c], fo[b:b + SHR, :])

        h2_cm.__exit__(None, None, None)

    nc.finalize()
    return nc


def _prep_inputs(hidden_states, attention_mask, W_pack, o_proj, gate_w, up_w,
                 down_w, ln1_w, ln2_w):
    """Slice/layout full inputs into 8 per-core input dicts."""
    hs = np.ascontiguousarray(np.asarray(hidden_states, dtype=np.float32)[0])
    hiddent = np.ascontiguousarray(hs.T)                      # [H, S]
    mask = np.asarray(attention_mask, dtype=np.float32)[0, 0]  # [S, S]
    masktd = np.stack([
        np.ascontiguousarray(mask[b * P:(b + 1) * P, b * P:(b + 1) * P].T)
        for b in range(NST)])                                  # [NST, P, P]
    W_pack = np.asarray(W_pack, dtype=np.float32)
    o_proj = np.asarray(o_proj, dtype=np.float32)
    gate_w = np.asarray(gate_w, dtype=np.float32)
    up_w = np.asarray(up_w, dtype=np.float32)
    down_w = np.asarray(down_w, dtype=np.float32)
    ln1 = np.ascontiguousarray(
        np.asarray(ln1_w, dtype=np.float32).reshape(NKC, P).T)  # [P, NKC]
    ln2 = np.asarray(ln2_w, dtype=np.float32).reshape(1, H)

    # intermediate split: 6 cores x 1408 + 2 cores x 1280 (padded to 1408)
    i_sizes = [1408] * 6 + [1280] * 2
    i_offs = np.cumsum([0] + i_sizes)

    in_maps = []
    for r in range(8):
        q0 = r * QKV_LOC
        wp = np.concatenate([
            W_pack[:, q0:q0 + QKV_LOC],
            W_pack[:, H + q0:H + q0 + QKV_LOC],
            W_pack[:, 2 * H + q0:2 * H + q0 + QKV_LOC]], axis=1)
        opl = o_proj[q0:q0 + QKV_LOC, :]
        io0, io1 = i_offs[r], i_offs[r + 1]
        isz = io1 - io0
        gl = np.zeros((H, I_LOC), np.float32)
        gl[:, :isz] = gate_w[:, io0:io1]
        ul = np.zeros((H, I_LOC), np.float32)
        ul[:, :isz] = up_w[:, io0:io1]
        dl = np.zeros((I_LOC, H), np.float32)
        dl[:isz, :] = down_w[io0:io1, :]
        hsh = np.stack([
            hs[c * CHS + r * SHR: c * CHS + (r + 1) * SHR, :]
            for c in range(NCH)])                              # [NCH, SHR, H]
        bf = ml_dtypes.bfloat16
        in_maps.append({
            "hiddent": hiddent,
            "hidshard": np.ascontiguousarray(hsh),
            "maskt": masktd,
            "wpack": np.ascontiguousarray(wp).astype(bf),
            "oproj": np.ascontiguousarray(opl).astype(bf),
            "gatew": gl.astype(bf),
            "upw": ul.astype(bf),
            "downw": dl.astype(bf),
            "ln1": ln1,
            "ln2": ln2,
        })
    return in_maps


def _assemble(results):
    """results[r]['out'] is [NCH, SHR, H]; reassemble [1, S, H]."""
    full = np.empty((S, H), np.float32)
    for r in range(8):
        o = results[r]["out"]
        for c in range(NCH):
            full[c * CHS + r * SHR: c * CHS + (r + 1) * SHR, :] = o[c]
    return full[None]


def _get_nc():
    if "nc" not in _CACHE:
        _CACHE["nc"] = _build()
    return _CACHE["nc"]


def kernel(**inputs):
    from concourse.bass_utils import run_bass_kernel_spmd
    nc = _get_nc()
    in_maps = _prep_inputs(**inputs)
    res = run_bass_kernel_spmd(nc, in_maps, core_ids=list(range(8)))
    return _assemble(res.results)


if __name__ == "__main__":
    rng = np.random.RandomState(0)
    ins = {
        "hidden_states": rng.randn(1, S, H).astype(np.float32),
        "attention_mask": np.where(
            np.tril(np.ones((S, S), bool)), 0.0,
            np.finfo(np.float32).min)[None, None].astype(np.float32),
        "W_pack": rng.randn(H, 3 * H).astype(np.float32) * 0.02,
        "o_proj": rng.randn(H, H).astype(np.float32) * 0.02,
        "gate_w": rng.randn(H, 11008).astype(np.float32) * 0.02,
        "up_w": rng.randn(H, 11008).astype(np.float32) * 0.02,
        "down_w": rng.randn(11008, H).astype(np.float32) * 0.02,
        "ln1_w": np.ones(H, np.float32),
        "ln2_w": np.ones(H, np.float32),
    }
    out = kernel(**ins)
    print("kernel output", out.shape, out.dtype, float(np.abs(out).mean()))


# revision 24
# speedup vs baseline: 12810.0059x; 1.0619x over previous
"""Baichuan transformer layer on 8 Trainium2 NeuronCores, tensor-parallel.

Sharding: heads (32 -> 4/core) and MLP intermediate (11008 -> ~1376/core,
padded to 1408) are split across 8 cores. W_pack/gate/up sharded column-wise,
o_proj/down row-wise. ReduceScatter after o_proj (sequence-sharded residual +
RMSNorm), AllGather of the normed bf16 activations, ReduceScatter after
down_proj. Final output is assembled from per-core sequence shards.

Dataflow keeps activations transposed ([hidden, seq]) for all matmuls; the
residual stream stays natural [seq, hidden] in fp32.
"""

import math
import os
import sys

sys.path.insert(0, "/opt/trn_rl_repo")

import ml_dtypes
import numpy as np

import concourse.bass as bass
import concourse.tile as tile
from concourse import bacc, mybir
from concourse.masks import make_identity

P = 128
S = 2048
H = 4096
NKC = H // P            # 32 hidden chunks
NH_LOC = 4              # heads per core
DH = 128
QKV_LOC = NH_LOC * DH   # 512
I_LOC = 1408            # padded local intermediate (11 * 128)
NIT = I_LOC // P        # 11
NST = S // P            # 16 seq tiles
NCH = 4                 # collective chunks
CHS = S // NCH          # 512 tokens per chunk
SHR = CHS // 8          # 64 rows per rank shard per chunk
EPS = 1e-6
SCALE = 1.0 / math.sqrt(DH)
BF = mybir.dt.bfloat16
F32 = mybir.dt.float32

COLL_DT = mybir.dt.bfloat16  # collective dtype (partials; residual math stays fp32)

_CACHE = {}


def _build():
    nc = bacc.Bacc("TRN2", target_bir_lowering=False, debug=False, num_devices=8)

    hiddent = nc.dram_tensor("hiddent", [H, S], F32, kind="ExternalInput")
    hidshard = nc.dram_tensor("hidshard", [NCH, SHR, H], F32, kind="ExternalInput")
    maskt = nc.dram_tensor("maskt", [NST, P, P], F32, kind="ExternalInput")
    wpack = nc.dram_tensor("wpack", [H, 3 * QKV_LOC], BF, kind="ExternalInput")
    oproj = nc.dram_tensor("oproj", [QKV_LOC, H], BF, kind="ExternalInput")
    gatew = nc.dram_tensor("gatew", [H, I_LOC], BF, kind="ExternalInput")
    upw = nc.dram_tensor("upw", [H, I_LOC], BF, kind="ExternalInput")
    downw = nc.dram_tensor("downw", [I_LOC, H], BF, kind="ExternalInput")
    ln1 = nc.dram_tensor("ln1", [P, NKC], F32, kind="ExternalInput")
    ln2 = nc.dram_tensor("ln2", [1, H], F32, kind="ExternalInput")
    out = nc.dram_tensor("out", [NCH, SHR, H], F32, kind="ExternalOutput")

    RG = [list(range(8))]
    MUL = mybir.AluOpType.mult
    ADD = mybir.AluOpType.add
    AF = mybir.ActivationFunctionType

    with tile.TileContext(nc) as tc:
      with tc.tile_pool(name="const", bufs=1) as cp, \
           tc.tile_pool(name="dram", bufs=1, space="DRAM") as dp:
        # ---- tiny constants (live whole kernel, ~1KB/partition) ----
        ln1sb = cp.tile([P, NKC], F32)
        nc.sync.dma_start(ln1sb[:], ln1[:])
        ident = cp.tile([P, P], F32)
        make_identity(nc, ident[:])
        ones_bf = cp.tile([P, 1], BF)
        nc.vector.memset(ones_bf[:], 1.0)
        epssb = cp.tile([P, 1], F32)
        nc.vector.memset(epssb[:], EPS)
        rsq_pcol = cp.tile([P, NST], F32)

        # dram scratch
        rsq_d = dp.tile([1, S], F32)
        rs1_in = [dp.tile([CHS, H], COLL_DT, name=f"rs1_in{c}")
                  for c in range(NCH)]
        rs1_out = dp.tile([NCH, SHR, H], COLL_DT)
        ag_in = dp.tile([NCH, H, SHR], BF)
        ag_out = [dp.tile([8 * H, SHR], BF, addr_space="Shared", name=f"ag_out{c}")
                  for c in range(NCH)]
        rs2_in = [dp.tile([CHS, H], COLL_DT, name=f"rs2_in{c}")
                  for c in range(NCH)]
        rs2_out = dp.tile([NCH, SHR, H], COLL_DT)
        had_d = dp.tile([NIT, P, S], BF)
        rec_d = dp.tile([NH_LOC, NST, P], F32)
        v_d = dp.tile([NST, P, QKV_LOC], BF)

        # long-lived pools with manual open/close (two-sided allocator)
        atp_cm = tc.tile_pool(name="atp", bufs=1)          # left: attnT p1-p4
        atp = atp_cm.__enter__()
        attnT = atp.tile([P, NH_LOC, S], BF)
        ht_cm = tc.tile_pool(name="ht", bufs=1)            # left: p1-p2
        htp = ht_cm.__enter__()
        ht = htp.tile([P, NKC, S], BF)

        # ==== phase 1: hiddenT load, sumsq, cast*ln1 ====
        with tc.tile_pool(name="hin", bufs=3) as hinp, \
             tc.tile_pool(name="sqp", bufs=2) as sqp, \
             tc.tile_pool(name="ssps", bufs=1, space="PSUM") as ssp, \
             tc.tile_pool(name="smal", bufs=1) as smp:
            ss = ssp.tile([1, S], F32)
            for k in range(NKC):
                hf = hinp.tile([P, S], F32, tag="hf")
                nc.sync.dma_start(hf[:], hiddent[k * P:(k + 1) * P, :])
                sq = sqp.tile([P, S], BF, tag="sq")
                nc.scalar.activation(sq[:], hf[:], AF.Square)
                for j in range(4):
                    nc.tensor.matmul(
                        ss[:, j * 512:(j + 1) * 512], ones_bf[:],
                        sq[:, j * 512:(j + 1) * 512],
                        start=(k == 0), stop=(k == NKC - 1))
                nc.vector.tensor_tensor(
                    ht[:, k, :], hf[:],
                    ln1sb[:, k:k + 1].to_broadcast((P, S)), MUL)
            # rsq = 1/sqrt(mean + eps)
            std = smp.tile([1, S], F32, tag="std")
            nc.scalar.activation(std[:], ss[:], AF.Sqrt,
                                 bias=epssb[:1, :], scale=1.0 / H)
            rsq = smp.tile([1, S], F32, tag="rsq")
            nc.vector.reciprocal(rsq[:], std[:])
            nc.sync.dma_start(rsq_d[:], rsq[:])
            nc.sync.dma_start(
                rsq_pcol[:], rsq_d.rearrange("o (n p) -> p (o n)", p=P))

        # right side: qkv outputs, live p2-p3
        qkv_cm = tc.tile_pool(name="qkv", bufs=1, side="right")
        qkvp = qkv_cm.__enter__()
        qT = qkvp.tile([P, NH_LOC, S], BF)
        kT = qkvp.tile([P, NH_LOC, S], BF)
        rsq_bc = qkvp.tile([P, S], BF)
        nc.gpsimd.dma_start(rsq_bc[:], rsq_d[:].to_broadcast((P, S)))

        # ==== phase 2: QKV projections ====
        with tc.tile_pool(name="wst", bufs=2) as wsp, \
             tc.tile_pool(name="qps", bufs=1, space="PSUM") as qpsp:
            for part in range(2):       # 0 = q, 1 = k
                dst = qT if part == 0 else kT
                for h in range(NH_LOC):
                    wcol = wsp.tile([P, NKC, P], BF, tag="wcol")
                    col0 = part * QKV_LOC + h * DH
                    nc.scalar.dma_start(
                        wcol[:],
                        wpack.rearrange("(k p) c -> p k c", p=P)
                        [:, :, col0:col0 + DH])
                    ps = [qpsp.tile([P, 512], F32, tag=f"qk{j}", name=f"qk{j}")
                          for j in range(4)]
                    for k in range(NKC):
                        for j in range(4):
                            nc.tensor.matmul(
                                ps[j][:], wcol[:, k, :],
                                ht[:, k, j * 512:(j + 1) * 512],
                                start=(k == 0), stop=(k == NKC - 1))
                    for j in range(4):
                        nc.vector.tensor_tensor(
                            dst[:, h, j * 512:(j + 1) * 512], ps[j][:],
                            rsq_bc[:, j * 512:(j + 1) * 512], MUL)
            # v in natural [s, d] layout (lhsT = hT chunk), staged to DRAM
            for vg in range(2):
                ps = [qpsp.tile([P, 512], F32, tag=f"qk{j}", name=f"vq{j}")
                      for j in range(4)] + \
                     [qpsp.tile([P, 512], F32, tag=f"v{j}", name=f"v{j}")
                      for j in range(4)]
                for k in range(NKC):
                    wv = wsp.tile([P, QKV_LOC], BF, tag="wv")
                    nc.scalar.dma_start(
                        wv[:], wpack[k * P:(k + 1) * P,
                                     2 * QKV_LOC:3 * QKV_LOC])
                    for sti in range(8):
                        st = vg * 8 + sti
                        nc.tensor.matmul(
                            ps[sti][:], ht[:, k, st * P:(st + 1) * P],
                            wv[:], start=(k == 0), stop=(k == NKC - 1))
                for sti in range(8):
                    st = vg * 8 + sti
                    vstg = wsp.tile([P, QKV_LOC], BF, tag="vstg")
                    nc.scalar.activation(
                        vstg[:], ps[sti][:], AF.Copy,
                        scale=rsq_pcol[:, st:st + 1])
                    nc.sync.dma_start(v_d[st], vstg[:])

        ht_cm.__exit__(None, None, None)   # free 128KB/part

        # prefetch o_proj weights during attention
        opj_cm = tc.tile_pool(name="opj", bufs=1)
        opp = opj_cm.__enter__()
        ow = opp.tile([P, NH_LOC, H], BF)
        for h in range(NH_LOC):
            nc.scalar.dma_start(ow[:, h, :], oproj[h * P:(h + 1) * P, :])

        # ==== phase 3: attention ====
        with tc.tile_pool(name="msk", bufs=1) as mkp, \
             tc.tile_pool(name="probs", bufs=4) as prp, \
             tc.tile_pool(name="vh", bufs=2) as vhp, \
             tc.tile_pool(name="scps", bufs=2, space="PSUM") as scp, \
             tc.tile_pool(name="atps", bufs=1, space="PSUM") as apsp, \
             tc.tile_pool(name="attmisc", bufs=2) as amp:
            masksb = mkp.tile([P, NST, P], F32)
            nc.sync.dma_start(masksb[:], maskt.rearrange("n k q -> k n q"))
            v_r = v_d.rearrange("st p c -> p st c")
            for h in range(NH_LOC):
                vh = vhp.tile([P, NST, DH], BF, tag="vh")
                nc.sync.dma_start(vh[:], v_r[:, :, h * DH:(h + 1) * DH])
                aps = apsp.tile([P, S], F32, tag="aps", name="aps")
                sps = apsp.tile([P, NST], F32, tag="sps", name="sps")
                for kb in range(NST):
                    q0 = kb * P
                    pt = prp.tile([P, S], BF, tag="probs", name="pt")
                    bnds = []
                    a = q0
                    while a < S:
                        b = min((a // 512 + 1) * 512, S)
                        bnds.append((a, b))
                        a = b
                    for (a, b) in bnds:
                        sc = scp.tile([P, 512], F32, tag="sc", name="sc")
                        nc.tensor.matmul(
                            sc[:, :b - a], kT[:, h, q0:q0 + P],
                            qT[:, h, a:b], start=True, stop=True)
                        if a == q0:
                            nc.vector.tensor_tensor(
                                sc[:, :P], sc[:, :P], masksb[:, kb, :], ADD)
                        nc.scalar.activation(
                            pt[:, a:b], sc[:, :b - a], AF.Exp, scale=SCALE)
                    for (a, b) in bnds:
                        nc.tensor.matmul(
                            aps[:, a:b], vh[:, kb, :], pt[:, a:b],
                            start=(kb == 0), stop=(kb == (b - 1) // P))
                    for qb in range(kb, NST):
                        # single bank shared by 16 accumulation chains:
                        # only the very first matmul may clear the bank
                        nc.tensor.matmul(
                            sps[:, qb:qb + 1], pt[:, qb * P:(qb + 1) * P],
                            ones_bf[:], start=(kb == 0 and qb == 0),
                            stop=(kb == qb), skip_group_check=True)
                rec = amp.tile([P, NST], F32, tag="rec")
                nc.vector.reciprocal(rec[:], sps[:])
                rtp = apsp.tile([NST, P], F32, tag="rtp", name="rtp")
                nc.tensor.transpose(rtp[:], rec[:], ident[:])
                rts = amp.tile([NST, P], F32, tag="rts")
                nc.scalar.copy(rts[:], rtp[:])
                nc.sync.dma_start(rec_d[h], rts[:])
                rbc = amp.tile([P, S], F32, tag="rbc")
                nc.gpsimd.dma_start(
                    rbc[:],
                    rec_d[h].rearrange("a b -> (a b)")[None, :]
                    .to_broadcast((P, S)))
                nc.vector.tensor_tensor(attnT[:, h, :], aps[:], rbc[:], MUL)

        qkv_cm.__exit__(None, None, None)

        # residual stream shards, live to the end (right side)
        h2_cm = tc.tile_pool(name="h2", bufs=1, side="right")
        h2p = h2_cm.__enter__()
        h2pk = [h2p.tile([P, H], F32, tag=f"h2_{j}", name=f"h2_{j}")
                for j in range(NCH // 2)]

        def h2sl(c):
            return h2pk[c // 2][(c % 2) * SHR:(c % 2) * SHR + SHR, :]

        # ==== phase 4: o_proj + per-chunk [RS1 -> norm -> AG] ====
        with tc.tile_pool(name="ops", bufs=1, space="PSUM") as opsp, \
             tc.tile_pool(name="ost", bufs=3) as ostp, \
             tc.tile_pool(name="chk", bufs=1) as chp:
            ln2bc = chp.tile([P, H], BF, tag="ln2bc")
            nc.gpsimd.dma_start(ln2bc[:], ln2[:].to_broadcast((P, H)))
            for st in range(NST):
                ps8 = [opsp.tile([P, 512], F32, tag=f"o{j}", name=f"o{j}")
                       for j in range(8)]
                for h in range(NH_LOC):
                    for j in range(8):
                        nc.tensor.matmul(
                            ps8[j][:], attnT[:, h, st * P:(st + 1) * P],
                            ow[:, h, j * 512:(j + 1) * 512],
                            start=(h == 0), stop=(h == NH_LOC - 1))
                osb = ostp.tile([P, H], COLL_DT, tag="osb")
                for j in range(8):
                    if j % 2 == 0:
                        nc.vector.tensor_copy(
                            osb[:, j * 512:(j + 1) * 512], ps8[j][:])
                    else:
                        nc.scalar.copy(
                            osb[:, j * 512:(j + 1) * 512], ps8[j][:])
                nc.sync.dma_start(
                    rs1_in[st // 4][(st % 4) * P:(st % 4 + 1) * P, :], osb[:])
                if st % 4 == 3:
                    c = st // 4
                    nc.gpsimd.collective_compute(
                        "ReduceScatter", ADD, replica_groups=RG,
                        ins=[rs1_in[c][:].opt()],
                        outs=[rs1_out[c].opt()])
            # per-chunk residual + rmsnorm + AllGather, emitted after the
            # o_proj loop so their RS1-waits don't block engine queues
            for c in range(NCH):
                b = (c % 2) * SHR
                h2c = h2sl(c)
                nc.sync.dma_start(h2c, hidshard[c])
                tmp = chp.tile([P, H], COLL_DT, tag="tmp")
                nc.sync.dma_start(tmp[b:b + SHR, :], rs1_out[c])
                nc.vector.tensor_tensor(h2c, h2c, tmp[b:b + SHR, :], ADD)
                sq2 = chp.tile([P, H], BF, tag="msh", name="sq2")
                nc.scalar.activation(sq2[b:b + SHR, :], h2c, AF.Square)
                var = chp.tile([P, 1], F32, tag="var")
                nc.vector.reduce_sum(var[b:b + SHR, :], sq2[b:b + SHR, :],
                                     axis=mybir.AxisListType.X)
                std2 = chp.tile([P, 1], F32, tag="std2")
                nc.scalar.activation(std2[b:b + SHR, :], var[b:b + SHR, :],
                                     AF.Sqrt, bias=epssb[b:b + SHR, :],
                                     scale=1.0 / H)
                rst = chp.tile([P, 1], F32, tag="rst")
                nc.vector.reciprocal(rst[b:b + SHR, :], std2[b:b + SHR, :])
                mtm = chp.tile([P, H], BF, tag="mtm")
                nc.scalar.activation(mtm[b:b + SHR, :], h2c, AF.Copy,
                                     scale=rst[b:b + SHR, :])
                msh = chp.tile([P, H], BF, tag="msh")
                nc.vector.tensor_tensor(msh[b:b + SHR, :], mtm[b:b + SHR, :],
                                        ln2bc[b:b + SHR, :], MUL)
                mts = chp.tile([P, NKC, SHR], BF, tag="mts")
                nc.sync.dma_start_transpose(mts[:], msh[b:b + SHR, :])
                nc.sync.dma_start(
                    ag_in[c].rearrange("(ks p) n -> p ks n", p=P), mts[:])
                nc.gpsimd.collective_compute(
                    "AllGather", mybir.AluOpType.bypass, replica_groups=RG,
                    ins=[ag_in[c].opt()], outs=[ag_out[c].opt()])

        opj_cm.__exit__(None, None, None)
        atp_cm.__exit__(None, None, None)

        mt_cm = tc.tile_pool(name="mt", bufs=1)
        mtp = mt_cm.__enter__()
        mT = mtp.tile([P, NKC, S], BF)

        # ==== phase 6: gate/up + silu (chunk-outer: overlap with AG pipeline) ====
        with tc.tile_pool(name="gst", bufs=2) as gsp, \
             tc.tile_pool(name="gwa", bufs=2) as gwap, \
             tc.tile_pool(name="gwb", bufs=2) as gwbp, \
             tc.tile_pool(name="gps", bufs=1, space="PSUM") as gpsp:
            gw_r = gatew.rearrange("(k p) c -> p k c", p=P)
            uw_r = upw.rearrange("(k p) c -> p k c", p=P)
            for c in range(NCH):
                c0 = c * CHS
                for r in range(8):
                    nc.sync.dma_start(
                        mT[:, :, c0 + r * SHR:c0 + (r + 1) * SHR],
                        ag_out[c][r * H:(r + 1) * H, :]
                        .rearrange("(ks p) n -> p ks n", p=P))
                for i in range(NIT):
                    gcol = gwap.tile([P, NKC, P], BF, tag="gcol")
                    nc.scalar.dma_start(gcol[:], gw_r[:, :, i * P:(i + 1) * P])
                    ucol = gwbp.tile([P, NKC, P], BF, tag="ucol")
                    nc.scalar.dma_start(ucol[:], uw_r[:, :, i * P:(i + 1) * P])
                    gp = gpsp.tile([P, 512], F32, tag=f"g{i % 4}", name="gp")
                    up = gpsp.tile([P, 512], F32, tag=f"u{i % 4}", name="up")
                    for k in range(NKC):
                        nc.tensor.matmul(
                            gp[:], gcol[:, k, :], mT[:, k, c0:c0 + CHS],
                            start=(k == 0), stop=(k == NKC - 1))
                        nc.tensor.matmul(
                            up[:], ucol[:, k, :], mT[:, k, c0:c0 + CHS],
                            start=(k == 0), stop=(k == NKC - 1))
                    gs = gsp.tile([P, CHS], BF, tag="gs")
                    us = gsp.tile([P, CHS], BF, tag="us")
                    nc.scalar.activation(gs[:], gp[:], AF.Silu)
                    nc.vector.tensor_copy(us[:], up[:])
                    hadt = gsp.tile([P, CHS], BF, tag="hadt")
                    nc.vector.tensor_tensor(hadt[:], gs[:], us[:], MUL)
                    nc.sync.dma_start(had_d[i][:, c0:c0 + CHS], hadt[:])

        mt_cm.__exit__(None, None, None)

        # ==== phase 7: down proj + RS2 ====
        with tc.tile_pool(name="dw", bufs=1) as dwp, \
             tc.tile_pool(name="dst", bufs=2) as dsp, \
             tc.tile_pool(name="hst", bufs=4) as hsp, \
             tc.tile_pool(name="dps", bufs=1, space="PSUM") as dpsp:
            dw = dwp.tile([P, NIT, H], BF)
            for i in range(NIT):
                nc.scalar.dma_start(dw[:, i, :], downw[i * P:(i + 1) * P, :])
            had_r = had_d.rearrange("i p s -> p i s")
            for st in range(NST):
                hads = hsp.tile([P, NIT, P], BF, tag="hads")
                nc.sync.dma_start(hads[:], had_r[:, :, st * P:(st + 1) * P])
                ps8 = [dpsp.tile([P, 512], F32, tag=f"d{j}", name=f"d{j}")
                       for j in range(8)]
                for i in range(NIT):
                    for j in range(8):
                        nc.tensor.matmul(
                            ps8[j][:], hads[:, i, :],
                            dw[:, i, j * 512:(j + 1) * 512],
                            start=(i == 0), stop=(i == NIT - 1))
                dsb = dsp.tile([P, H], COLL_DT, tag="dsb")
                for j in range(8):
                    if j % 2 == 0:
                        nc.vector.tensor_copy(
                            dsb[:, j * 512:(j + 1) * 512], ps8[j][:])
                    else:
                        nc.scalar.copy(
                            dsb[:, j * 512:(j + 1) * 512], ps8[j][:])
                nc.sync.dma_start(
                    rs2_in[st // 4][(st % 4) * P:(st % 4 + 1) * P, :], dsb[:])
                if st % 4 == 3:
                    c = st // 4
                    nc.gpsimd.collective_compute(
                        "ReduceScatter", ADD, replica_groups=RG,
                        ins=[rs2_in[c][:].opt()],
                        outs=[rs2_out[c].opt()])
            # ==== phase 8: final residual ====
            with tc.tile_pool(name="fin", bufs=1) as fpp:
                for c in range(NCH):
                    b = (c % 2) * SHR
                    f1 = fpp.tile([P, H], COLL_DT, tag="f1")
                    nc.sync.dma_start(f1[b:b + SHR, :], rs2_out[c])
                    fo = fpp.tile([P, H], F32, tag="fo")
                    nc.vector.tensor_tensor(fo[b:b + SHR, :], f1[b:b + SHR, :],
                                            h2sl(c), ADD)
                    nc.sync.dma_start(out[c], fo[b:b + SHR, :])

        h2_cm.__exit__(None, None, None)

    nc.finalize()
    return nc


def _prep_inputs(hidden_states, attention_mask, W_pack, o_proj, gate_w, up_w,
                 down_w, ln1_w, ln2_w):
    """Slice/layout full inputs into 8 per-core input dicts."""
    hs = np.ascontiguousarray(np.asarray(hidden_states, dtype=np.float32)[0])
    hiddent = np.ascontiguousarray(hs.T)                      # [H, S]
    mask = np.asarray(attention_mask, dtype=np.float32)[0, 0]  # [S, S]
    masktd = np.stack([
        np.ascontiguousarray(mask[b * P:(b + 1) * P, b * P:(b + 1) * P].T)
        for b in range(NST)])                                  # [NST, P, P]
    W_pack = np.asarray(W_pack, dtype=np.float32)
    o_proj = np.asarray(o_proj, dtype=np.float32)
    gate_w = np.asarray(gate_w, dtype=np.float32)
    up_w = np.asarray(up_w, dtype=np.float32)
    down_w = np.asarray(down_w, dtype=np.float32)
    ln1 = np.ascontiguousarray(
        np.asarray(ln1_w, dtype=np.float32).reshape(NKC, P).T)  # [P, NKC]
    ln2 = np.asarray(ln2_w, dtype=np.float32).reshape(1, H)

    # intermediate split: 6 cores x 1408 + 2 cores x 1280 (padded to 1408)
    i_sizes = [1408] * 6 + [1280] * 2
    i_offs = np.cumsum([0] + i_sizes)

    in_maps = []
    for r in range(8):
        q0 = r * QKV_LOC
        wp = np.concatenate([
            W_pack[:, q0:q0 + QKV_LOC],
            W_pack[:, H + q0:H + q0 + QKV_LOC],
            W_pack[:, 2 * H + q0:2 * H + q0 + QKV_LOC]], axis=1)
        opl = o_proj[q0:q0 + QKV_LOC, :]
        io0, io1 = i_offs[r], i_offs[r + 1]
        isz = io1 - io0
        gl = np.zeros((H, I_LOC), np.float32)
        gl[:, :isz] = gate_w[:, io0:io1]
        ul = np.zeros((H, I_LOC), np.float32)
        ul[:, :isz] = up_w[:, io0:io1]
        dl = np.zeros((I_LOC, H), np.float32)
        dl[:isz, :] = down_w[io0:io1, :]
        hsh = np.stack([
            hs[c * CHS + r * SHR: c * CHS + (r + 1) * SHR, :]
            for c in range(NCH)])                              # [NCH, SHR, H]
        bf = ml_dtypes.bfloat16
        in_maps.append({
            "hiddent": hiddent,
            "hidshard": np.ascontiguousarray(hsh),
            "maskt": masktd,
            "wpack": np.ascontiguousarray(wp).astype(bf),
            "oproj": np.ascontiguousarray(opl).astype(bf),
            "gatew": gl.astype(bf),
            "upw": ul.astype(bf),
            "downw": dl.astype(bf),
            "ln1": ln1,
            "ln2": ln2,
        })
    return in_maps


def _assemble(results):
    """results[r]['out'] is [NCH, SHR, H]; reassemble [1, S, H]."""
    full = np.empty((S, H), np.float32)
    for r in range(8):
        o = results[r]["out"]
        for c in range(NCH):
            full[c * CHS + r * SHR: c * CHS + (r + 1) * SHR, :] = o[c]
    return full[None]


def _get_nc():
    if "nc" not in _CACHE:
        _CACHE["nc"] = _build()
    return _CACHE["nc"]


def kernel(**inputs):
    from concourse.bass_utils import run_bass_kernel_spmd
    nc = _get_nc()
    in_maps = _prep_inputs(**inputs)
    res = run_bass_kernel_spmd(nc, in_maps, core_ids=list(range(8)))
    return _assemble(res.results)


if __name__ == "__main__":
    rng = np.random.RandomState(0)
    ins = {
        "hidden_states": rng.randn(1, S, H).astype(np.float32),
        "attention_mask": np.where(
            np.tril(np.ones((S, S), bool)), 0.0,
            np.finfo(np.float32).min)[None, None].astype(np.float32),
        "W_pack": rng.randn(H, 3 * H).astype(np.float32) * 0.02,
        "o_proj": rng.randn(H, H).astype(np.float32) * 0.02,
        "gate_w": rng.randn(H, 11008).astype(np.float32) * 0.02,
        "up_w": rng.randn(H, 11008).astype(np.float32) * 0.02,
        "down_w": rng.randn(11008, H).astype(np.float32) * 0.02,
        "ln1_w": np.ones(H, np.float32),
        "ln2_w": np.ones(H, np.float32),
    }
    out = kernel(**ins)
    print("kernel output", out.shape, out.dtype, float(np.abs(out).mean()))
